# revision 80
# baseline (speedup 1.0000x reference)
"""Trainium2 Bass kernel for nn_ByteGridModel (dense_cnn).

Sharding: pure data-parallel over batch B=8 -> 8 cores, one batch item per
core, no collectives. Weights replicated (streamed per layer, double
buffered).

Per-core layout: channels on partitions, h = [H=512 -> 4x128, S=256] fp32
resident in SBUF.

Per layer (engine assignment tuned against the CoreSim cost model):
  - rmsnorm: ACT square -> fp32r ones-matmul partition reduction -> ACT sqrt
    -> DVE reciprocal -> fp32r broadcast matmul -> ACT copy to SBUF.
  - per-channel 16x16 mixers: single DVE/Pool broadcast-AP product per
    c-tile with the shared axis innermost (j for local, i for global) so all
    operands have packed 2-byte last dims -> DVE 2x mode. The global mixer's
    v is written per-block transposed by the norm-mul (free: the norm-mul
    runs at 1x regardless since h is fp32). Reduction via 16 identity
    matmuls into PSUM; h updates on Pool.
  - GLU MLP: bf16 PE matmuls (Wv/Wg/Wo), Silu on ACT, gate-mul on Pool.
  - rms weights / alphas are folded into the mixer/GLU weights on host.
"""

import numpy as np
import ml_dtypes

import concourse.bacc as bacc
import concourse.bass as bass
import concourse.tile as tile
import concourse.mybir as mybir
from concourse.bass_utils import run_bass_kernel_spmd

B, S, H, GLU, VOC, L, CIN, BLK = 8, 256, 512, 1024, 256, 24, 320, 16
EPS = 1e-5
NT = H // 128  # 4 channel tiles
GT = GLU // 128  # 8 glu tiles

F32 = mybir.dt.float32
F32R = mybir.dt.float32r
BF16 = mybir.dt.bfloat16
FP8 = mybir.dt.float8e4
MULT = mybir.AluOpType.mult
ADD = mybir.AluOpType.add
AF = mybir.ActivationFunctionType
DR = mybir.MatmulPerfMode.DoubleRow

WSC = 64.0  # fp8 weight scale for wv/wg/wo (avoids e4m3 subnormals)
GSC = 8.0  # fp8 gate activation scale

# mixer product halves run on Pool (rest on DVE): (tile, half) pairs
POOL_HALVES = ((0, 1), (1, 1), (2, 1))
CHAIN_ORDER = (0, 1, 2, 3)

# warm-keeper matmul counts (fill PE idle windows to hold the p-state ramp)
WARM = {"rms": 0, "bcast": 0, "mixer": 0, "glu_p": 0, "glu_o": 0}

_PROG_CACHE = {}


def _bview(base, doff, free_dims):
    """View of a 2D sbuf AP with custom (possibly broadcast) free dims."""
    return bass.AP(
        tensor=base.tensor,
        offset=base.offset + doff,
        ap=[list(base.ap[0])] + [list(d) for d in free_dims],
    )


def build_program(n_layers=L, sim_compat=False):
    nc = bacc.Bacc("TRN2")

    x_d = nc.dram_tensor("x", [384, S], F32R, kind="ExternalInput")
    stw_d = nc.dram_tensor("stem_wT", [384, H], F32R, kind="ExternalInput")
    wv_d = nc.dram_tensor("wvT", [n_layers, H, GLU], FP8, kind="ExternalInput")
    wg_d = nc.dram_tensor("wgT", [n_layers, H, GLU], FP8, kind="ExternalInput")
    wo_d = nc.dram_tensor("woT", [n_layers, GLU, H], FP8, kind="ExternalInput")
    wl_d = nc.dram_tensor("wl", [n_layers, H, 256], BF16, kind="ExternalInput")
    wm_d = nc.dram_tensor("wm", [n_layers, H, 256], BF16, kind="ExternalInput")
    hw_d = nc.dram_tensor("headT", [H, VOC], BF16, kind="ExternalInput")
    id_d = nc.dram_tensor("ident", [128, 128], BF16, kind="ExternalInput")
    idr_d = nc.dram_tensor("ident_r", [128, 128], F32R, kind="ExternalInput")
    id5_d = nc.dram_tensor("ident512", [128, 128], F32R, kind="ExternalInput")
    ones_d = nc.dram_tensor("ones_k", [128, 1], F32R, kind="ExternalInput")
    onesr_d = nc.dram_tensor("ones_m", [1, 128], F32R, kind="ExternalInput")
    out_d = nc.dram_tensor("out", [VOC, S], F32, kind="ExternalOutput")

    from contextlib import ExitStack

    with tile.TileContext(nc) as tc, ExitStack() as ctx:
        singles = ctx.enter_context(tc.tile_pool(name="singles", bufs=1))
        wpool = ctx.enter_context(tc.tile_pool(name="wpool", bufs=2))
        hpool = ctx.enter_context(tc.tile_pool(name="hpool", bufs=1))
        npool = ctx.enter_context(tc.tile_pool(name="npool", bufs=2))
        apool = ctx.enter_context(tc.tile_pool(name="apool", bufs=2))
        ppool = ctx.enter_context(tc.tile_pool(name="ppool", bufs=5))
        gpool = ctx.enter_context(tc.tile_pool(name="gpool", bufs=2))
        ps_n = ctx.enter_context(tc.tile_pool(name="ps_n", bufs=1, space="PSUM"))
        ps_m = ctx.enter_context(tc.tile_pool(name="ps_m", bufs=2, space="PSUM"))
        ps_g = ctx.enter_context(tc.tile_pool(name="ps_g", bufs=3, space="PSUM"))
        ps_o = ctx.enter_context(tc.tile_pool(name="ps_o", bufs=2, space="PSUM"))

        # ---- constants / stem operands ----
        ident = singles.tile([128, 128], BF16, tag="ident")
        nc.sync.dma_start(out=ident, in_=id_d[:])
        identr_st = singles.tile([128, 128], F32R, tag="identr_st")
        nc.sync.dma_start(out=identr_st, in_=idr_d[:])
        identr = singles.tile([128, 128], F32R, tag="identr")
        id512_st = singles.tile([128, 128], F32R, tag="id512_st")
        nc.sync.dma_start(out=id512_st, in_=id5_d[:])
        id512 = singles.tile([128, 128], F32R, tag="id512")
        ones_k_st = singles.tile([128, 1], F32R, tag="ones_k_st")
        nc.sync.dma_start(out=ones_k_st, in_=ones_d[:])
        ones_k = singles.tile([128, 1], F32R, tag="ones_k")
        ones_m_st = singles.tile([1, 128], F32R, tag="ones_m_st")
        nc.sync.dma_start(out=ones_m_st, in_=onesr_d[:])
        ones_m = singles.tile([1, 128], F32R, tag="ones_m")
        eps_sb = singles.tile([1, 1], F32, tag="eps")
        nc.vector.memset(eps_sb, float(EPS))
        dmy = singles.tile([1, 1], F32, tag="dmy")

        def preload_table(func, dep=None):
            # dummy op to hoist the ACT table reload off the critical path;
            # dep pins the earliest-start so the scheduler overlaps the load
            nc.scalar.activation(dmy, eps_sb if dep is None else dep, func)

        x_st = singles.tile([128, 3, S], F32R, tag="x_st")
        nc.sync.dma_start(out=x_st, in_=x_d[:].rearrange("(t p) s -> p t s", p=128))
        x_sb = singles.tile([128, 3, S], F32R, tag="x")
        stw_st = singles.tile([128, 3, H], F32R, tag="stw_st")
        nc.sync.dma_start(out=stw_st, in_=stw_d[:].rearrange("(t p) s -> p t s", p=128))
        stw_sb = singles.tile([128, 3, H], F32R, tag="stw")

        # Route fp32r matmul operands through a DVE copy so each matmul's
        # operand has an engine writer (a matmul can carry only one
        # cross-engine wait through walrus codegen). Touch bf16 weight DMAs
        # with ldweights for the same reason.
        with nc.allow_low_precision(reason="fp32r staging copies"):
            nc.vector.tensor_copy(out=ones_k, in_=ones_k_st)
            nc.vector.tensor_copy(out=ones_m, in_=ones_m_st)
            nc.vector.tensor_copy(out=x_sb, in_=x_st)
            nc.vector.tensor_copy(out=stw_sb, in_=stw_st)
            nc.vector.tensor_copy(out=identr, in_=identr_st)
            nc.vector.tensor_copy(out=id512, in_=id512_st)
        nc.tensor.ldweights(ident[:, 0:128])

        # ---- h tiles (resident, f32r so identity matmuls can stream them) ----
        h = [
            hpool.tile([128, S], F32R, tag=f"h{t}", name=f"h{t}") for t in range(NT)
        ]

        # warm-keeper: dummy matmuls into the spare region of the broadcast
        # PSUM bank; they only run when PE is otherwise idle and keep the
        # p-state ramp alive through dependency stalls
        warm_dest = [None]

        def warm(n):
            if warm_dest[0] is None or n == 0:
                return
            for _ in range(n):
                nc.tensor.matmul(
                    warm_dest[0],
                    ident[:, 0:64],
                    ident[:],
                    start=True,
                    stop=True,
                    skip_group_check=True,
                )

        # ---- stem: h = stem_w @ x ----
        for t in range(NT):
            pst = ps_o.tile([128, S], F32, tag="po")
            for kt in range(3):
                nc.tensor.matmul(
                    pst,
                    stw_sb[:, kt, t * 128 : (t + 1) * 128],
                    x_sb[:, kt, :],
                    start=(kt == 0),
                    stop=(kt == 2),
                )
            with nc.allow_low_precision(reason="f32r h tiles"):
                nc.vector.tensor_copy(out=h[t], in_=pst)

        def rms_sbuf(srcs=None, jp=False):
            """Returns SBUF [128, S] f32 broadcast of 1/sqrt(mean(h^2)+eps).

            srcs: optional per-tile (psum_acc, scale) pairs holding
            scale*acc == h_new; squares then read the PSUM accs directly so
            the rms chain starts before the h copy-back lands.
            jp: srcs accs are (j,p)-ordered; h reads must match that order
            (rb then comes back (j,p)-ordered too).
            """
            msrb = ps_n.tile([128, 512], F32, tag="msrb")
            warm(WARM["rms"])
            ms = msrb[0:1, 256:512]
            for t in range(NT):
                sq = apool.tile([128, S], F32R, tag="sq")
                # t>=2 reads the PSUM acc directly (their banks' next writers
                # come after these squares in program order, so no WAR
                # circularity); t<2 reads h after the early copy-back
                hv = h[t][:] if not jp else _bview(h[t][:], 0, [[1, 16], [16, 16]])
                if srcs is not None and t >= 2:
                    src, sc = srcs[t]
                    with nc.allow_low_precision(reason="fp32r squares"):
                        nc.scalar.activation(sq, src, AF.Square, scale=sc)
                elif t < 2:
                    nc.scalar.square(sq, hv)
                else:
                    with nc.allow_low_precision(reason="fp32r squares"):
                        nc.vector.tensor_tensor(out=sq, in0=hv, in1=hv, op=MULT)
                nc.tensor.matmul(
                    ms,
                    ones_k[:, 0:1],
                    sq[:],
                    start=(t == 0),
                    stop=(t == NT - 1),
                )
            stdv = npool.tile([1, S], F32R, tag="stdv")
            with nc.allow_low_precision(reason="fp32r stdv for broadcast matmul"):
                nc.scalar.activation(
                    stdv, ms, AF.Sqrt, bias=eps_sb[0:1, 0:1], scale=1.0 / H
                )
            warm(WARM["bcast"])
            sdb = msrb[:, 0:256]
            nc.tensor.matmul(
                sdb,
                ones_m[0:1, :],
                stdv[:],
                start=True,
                stop=True,
            )
            rb = npool.tile([128, S], F32, tag="rbs")
            nc.vector.reciprocal(rb, sdb)
            return rb

        def mixer(wbase_sb, glob, prod_eng_pool, srcs=None):
            """One mixer sublayer over all 4 c-tiles. Returns PSUM accs.

            local (glob=False): out[c,i,p] = sum_j Wl[c,(p,j)] u[c,(i,j)]
            global (glob=True):  out[c,p,j] = sum_i Wm[c,(p,i)] v[c,(i,j)]
            """
            rb = rms_sbuf(srcs)
            uns = []
            for t in range(NT):
                un = apool.tile([128, S], BF16, tag=f"u{t}")
                if not glob:
                    # u in natural (i,j) order
                    nc.gpsimd.tensor_tensor(out=un, in0=h[t], in1=rb, op=MULT)
                else:
                    # v written per-block transposed: vT[c, 16j+i]
                    nc.gpsimd.tensor_tensor(
                        out=_bview(un[:], 0, [[1, 16], [16, 16]]),
                        in0=_bview(h[t][:], 0, [[16, 16], [1, 16]]),
                        in1=_bview(rb[:], 0, [[16, 16], [1, 16]]),
                        op=MULT,
                    )
                uns.append(un)
            # products split into halves along the innermost (reduced) axis;
            # each half its own tile so identity matmuls start after half A.
            # local: out (i,p,j); in0 u (i,p,j); in1 wl (i,p,j)
            # glob:  out (j,p,i); in0 vT (j,p,i); in1 wm (j,p,i)
            prods = []
            for t in range(NT):
                wbase = wbase_sb[:, t, :]
                halves = []
                for hf in range(2):
                    ph = ppool.tile([128, 2048], BF16, tag=f"prod{hf}")
                    eng = nc.gpsimd if (t, hf) in prod_eng_pool else nc.vector
                    eng.tensor_tensor(
                        out=_bview(ph[:], 0, [[128, 16], [8, 16], [1, 8]]),
                        in0=_bview(uns[t][:], 8 * hf, [[16, 16], [0, 16], [1, 8]]),
                        in1=_bview(wbase, 8 * hf, [[0, 16], [16, 16], [1, 8]]),
                        op=MULT,
                    )
                    halves.append(ph)
                prods.append(halves)
            warm(WARM["mixer"])
            accs = [None] * NT
            # chains ordered so the tile fed by Pool's last product comes
            # last; t3 (fully DVE-fed) is ready before t2's Pool half
            for t in CHAIN_ORDER:
                acc = ps_m.tile([128, S], F32, tag="macc")
                # fold h into the PSUM chain via an f32r identity matmul so
                # the update is a pure PSUM->SBUF copy (GPSIMD can't read
                # PSUM on HW); h streamed in the same (X,p) order as acc
                hr = h[t][:]
                nc.tensor.matmul(
                    acc,
                    identr[:],
                    hr if not glob else _bview(hr, 0, [[1, 16], [16, 16]]),
                    start=True,
                    stop=False,
                )
                for q in range(16):
                    hf, qq = divmod(q, 8)
                    nc.tensor.matmul(
                        acc,
                        ident[:],
                        _bview(prods[t][hf][:], qq, [[128, 16], [8, 16]]),
                        start=False,
                        stop=(q == 15),
                    )
                # h copy-back: (i,p) is natural s' order for local; (j,p)
                # for global, where h[c, 16p+j] is viewed as (j,p)
                hv = h[t][:] if not glob else _bview(h[t][:], 0, [[1, 16], [16, 16]])
                if t < 2:
                    with nc.allow_low_precision(reason="f32r h tiles"):
                        nc.scalar.activation(hv, acc, AF.Identity)
                else:
                    with nc.allow_low_precision(reason="f32r h tiles"):
                        nc.vector.tensor_copy(out=hv, in_=acc)
                accs[t] = acc
            return accs

        last_accs = None  # previous sublayer's (psum_acc, scale) per tile
        for l in range(n_layers):
            wv_sb = wpool.tile([128, NT, GLU], FP8, tag="wv")
            nc.sync.dma_start(
                out=wv_sb, in_=wv_d[l].rearrange("(t p) o -> p t o", p=128)
            )
            wg_sb = wpool.tile([128, NT, GLU], FP8, tag="wg")
            nc.sync.dma_start(
                out=wg_sb, in_=wg_d[l].rearrange("(t p) o -> p t o", p=128)
            )
            wo_sb = wpool.tile([128, GT, H], FP8, tag="wo")
            nc.sync.dma_start(
                out=wo_sb, in_=wo_d[l].rearrange("(t p) c -> p t c", p=128)
            )
            wl_sb = wpool.tile([128, NT, 256], BF16, tag="wl")
            nc.sync.dma_start(
                out=wl_sb, in_=wl_d[l].rearrange("(t p) q -> p t q", p=128)
            )
            wm_sb = wpool.tile([128, NT, 256], BF16, tag="wm")
            nc.sync.dma_start(
                out=wm_sb, in_=wm_d[l].rearrange("(t p) q -> p t q", p=128)
            )
            nc.tensor.ldweights(wv_sb[:, 0, 0:128])
            nc.tensor.ldweights(wg_sb[:, 0, 0:128])
            nc.tensor.ldweights(wo_sb[:, 0, 0:128])

            # ---------- local mixer: out[c,i,p] = sum_j Wl[c,p,j] u[c,i,j]
            local_accs = mixer(
                wl_sb, glob=False, prod_eng_pool=POOL_HALVES, srcs=last_accs
            )
            # ---------- global mixer: out[c,p,j] = sum_i Wm[c,p,i] v[c,i,j]
            global_accs = mixer(
                wm_sb, glob=True, prod_eng_pool=POOL_HALVES,
                srcs=[(a, 1.0) for a in local_accs],
            )

            # ---------- GLU MLP (fp8 DoubleRow matmuls; weights scaled by WSC)
            # global accs are (j,p)-ordered, so rb comes back (j,p)-ordered:
            # rb[16j+p] holds the value for position s=16p+j
            rb = rms_sbuf([(a, 1.0) for a in global_accs], jp=True)
            # wn pairs: [128, 2, S] fp8 per k-tile pair for DoubleRow rhs;
            # written per-position via (p,j) views to undo rb's ordering
            wn = []
            for q in range(NT // 2):
                wp = apool.tile([128, 2, S], FP8, tag=f"wn{q}")
                for r in range(2):
                    nc.gpsimd.tensor_tensor(
                        out=_bview(wp[:, r, :], 0, [[16, 16], [1, 16]]),
                        in0=_bview(h[2 * q + r][:], 0, [[16, 16], [1, 16]]),
                        in1=_bview(rb[:], 0, [[1, 16], [16, 16]]),
                        op=MULT,
                    )
                wn.append(wp)
            warm(WARM["glu_p"])
            gts = []
            for ot in range(GT):
                p1 = ps_g.tile([128, S], F32, tag="pg")
                for q in range(NT // 2):
                    nc.tensor.matmul(
                        p1,
                        wv_sb[:, 2 * q : 2 * q + 2, ot * 128 : (ot + 1) * 128],
                        wn[q][:],
                        start=(q == 0),
                        stop=(q == NT // 2 - 1),
                        perf_mode=DR,
                    )
                s1 = apool.tile([128, S], FP8, tag="s1")
                if sim_compat:
                    # CoreSim has no Silu: emulate with Sigmoid + extra mul
                    sg = apool.tile([128, S], BF16, tag="sg")
                    nc.scalar.activation(sg, p1, AF.Sigmoid, scale=1.0 / WSC)
                    nc.vector.scalar_tensor_tensor(
                        out=s1, in0=p1, scalar=1.0 / WSC, in1=sg,
                        op0=MULT, op1=MULT,
                    )
                else:
                    nc.scalar.activation(s1, p1, AF.Silu, scale=1.0 / WSC)
                p3 = ps_g.tile([128, S], F32, tag="pg")
                for q in range(NT // 2):
                    nc.tensor.matmul(
                        p3,
                        wg_sb[:, 2 * q : 2 * q + 2, ot * 128 : (ot + 1) * 128],
                        wn[q][:],
                        start=(q == 0),
                        stop=(q == NT // 2 - 1),
                        perf_mode=DR,
                    )
                # gt = (p3 / WSC * GSC) * s1, stored fp8 (scaled by GSC)
                qg, rg_ = divmod(ot, 2)
                if rg_ == 0:
                    gp = gpool.tile([128, 2, S], FP8, tag=f"g{qg}")
                    gts.append(gp)
                nc.vector.scalar_tensor_tensor(
                    out=gts[qg][:, rg_, :], in0=p3, scalar=GSC / WSC, in1=s1,
                    op0=MULT, op1=MULT,
                )
            warm(WARM["glu_o"])
            last_accs = []
            for t in range(NT):
                po = ps_o.tile([128, S], F32, tag="po")
                # acc = (WSC*GSC)*h + (WSC*GSC)*update via a scaled identity;
                # h update is then a pure ACT copy with scale 1/(WSC*GSC)
                nc.tensor.matmul(
                    po,
                    id512[:],
                    h[t][:],
                    start=True,
                    stop=False,
                )
                for q in range(GT // 2):
                    nc.tensor.matmul(
                        po,
                        wo_sb[:, 2 * q : 2 * q + 2, t * 128 : (t + 1) * 128],
                        gts[q][:],
                        start=False,
                        stop=(q == GT // 2 - 1),
                        perf_mode=DR,
                    )
                if t < 2:
                    with nc.allow_low_precision(reason="f32r h tiles"):
                        nc.scalar.activation(
                            h[t], po, AF.Identity, scale=1.0 / (WSC * GSC)
                        )
                else:
                    with nc.allow_low_precision(reason="f32r h tiles"):
                        nc.vector.tensor_scalar_mul(
                            out=h[t], in0=po, scalar1=1.0 / (WSC * GSC)
                        )
                last_accs.append((po, 1.0 / (WSC * GSC)))

        # ---------- head ----------
        hw_sb = singles.tile([128, NT, VOC], BF16, tag="hw")
        nc.sync.dma_start(out=hw_sb, in_=hw_d.rearrange("(t p) v -> p t v", p=128))
        nc.tensor.ldweights(hw_sb[:, 0, 0:128])
        rb = rms_sbuf(last_accs)
        nrm = []
        for t in range(NT):
            n_ = apool.tile([128, S], BF16, tag=f"wn{t}")
            nc.vector.tensor_tensor(out=n_, in0=h[t], in1=rb, op=MULT)
            nrm.append(n_)
        for mc in range(VOC // 128):
            po = ps_o.tile([128, S], F32, tag="po")
            for kt in range(NT):
                nc.tensor.matmul(
                    po,
                    hw_sb[:, kt, mc * 128 : (mc + 1) * 128],
                    nrm[kt][:],
                    start=(kt == 0),
                    stop=(kt == NT - 1),
                )
            ot_sb = apool.tile([128, S], F32, tag="osb")
            nc.vector.tensor_copy(out=ot_sb, in_=po)
            nc.sync.dma_start(out=out_d[mc * 128 : (mc + 1) * 128, :], in_=ot_sb)

    nc.compile()
    return nc


def _prep_inputs(inputs, n_layers=L):
    """Host-side weight folding + layout prep. Returns dict of np arrays."""
    f = lambda k: np.asarray(inputs[k], dtype=np.float32)
    x = f("x")
    stem_w = f("stem_w")  # [H, CIN]
    rl, rg, rf = f("rms_local"), f("rms_global"), f("rms_ffn")
    al, ag, am = f("alpha_local"), f("alpha_global"), f("alpha_mlp")
    w_local, w_global = f("w_local"), f("w_global")  # [L, H, BLK, BLK]
    wv, wg, wo = f("wv"), f("wg"), f("wo")
    head_rms, head_w = f("head_rms"), f("head_w")
    hls = np.float32(np.asarray(inputs["head_logit_scale"]))

    bf = ml_dtypes.bfloat16
    nl = n_layers

    # local: fold alpha_local * rms_local[c] into Wl[c,p,j]; layout [c, 16p+j]
    wl_h = (w_local[:nl] * al[:nl, None, None, None] * rl[:nl, :, None, None]).reshape(
        nl, H, 256
    )
    # global: Wg[c,p,i]; layout [c, 16p+i]
    wm_h = (w_global[:nl] * ag[:nl, None, None, None] * rg[:nl, :, None, None]).reshape(
        nl, H, 256
    )
    # GLU: fold rms_ffn into wv/wg columns; alpha_mlp into wo
    wvT = np.ascontiguousarray(
        np.transpose(wv[:nl] * rf[:nl, None, :], (0, 2, 1))
    )  # [L, H, GLU]
    wgT = np.ascontiguousarray(np.transpose(wg[:nl] * rf[:nl, None, :], (0, 2, 1)))
    woT = np.ascontiguousarray(
        np.transpose(wo[:nl] * am[:nl, None, None], (0, 2, 1))
    )  # [L, GLU, H]
    headT = np.ascontiguousarray((head_w * head_rms[None, :] * hls).T)  # [H, VOC]

    stw_pad = np.zeros((384, H), np.float32)
    stw_pad[:CIN] = stem_w.T
    f8 = ml_dtypes.float8_e4m3
    common = {
        "stem_wT": stw_pad,  # [384, H] zero-padded
        "wvT": (wvT * 64.0).astype(f8),
        "wgT": (wgT * 64.0).astype(f8),
        "woT": (woT * 64.0).astype(f8),
        "wl": wl_h.astype(bf),
        "wm": wm_h.astype(bf),
        "headT": headT.astype(bf),
        "ident": np.eye(128, dtype=bf),
        "ident_r": np.eye(128, dtype=np.float32),
        "ident512": (512.0 * np.eye(128)).astype(np.float32),
        "ones_k": np.ones((128, 1), np.float32),
        "ones_m": np.ones((1, 128), np.float32),
    }
    per_core = []
    for b in range(B):
        xp = np.zeros((384, S), np.float32)
        xp[:CIN] = x[b, :, 0, :]
        per_core.append(dict(common, x=xp))
    return per_core


def run(inputs, n_layers=L, trace=False):
    key = n_layers
    if key not in _PROG_CACHE:
        _PROG_CACHE[key] = build_program(n_layers)
    nc = _PROG_CACHE[key]
    in_maps = _prep_inputs(inputs, n_layers)
    res = run_bass_kernel_spmd(nc, in_maps, core_ids=list(range(B)), trace=trace)
    out = np.stack([r["out"] for r in res.results])  # [B, VOC, S]
    return out[:, :, None, :].astype(np.float32), res


def kernel(**inputs):
    out, _ = run(inputs, L, trace=False)
    return out


# revision 82
# speedup vs baseline: 1.4540x; 1.4540x over previous
"""Trainium2 Bass kernel for nn_ByteGridModel (dense_cnn).

Sharding: pure data-parallel over batch B=8 -> 8 cores, one batch item per
core, no collectives. Weights replicated (streamed per layer, double
buffered).

Per-core layout: channels on partitions, h = [H=512 -> 4x128, S=256] f32r
resident in SBUF (f32r so identity matmuls can stream h into PSUM chains).

Per layer (engine assignment tuned against the CoreSim cost model; GPSIMD
never touches PSUM - the HW BIR verifier rejects that):
  - rmsnorm: squares on ACT (read previous sublayer's PSUM accs directly
    where the bank rotation allows, so the chain starts before h lands) ->
    f32r ones-matmul partition reduction -> ACT sqrt -> f32r broadcast
    matmul -> DVE reciprocal into SBUF.
  - per-channel 16x16 mixers: norm-muls on Pool (the global mixer's v is
    written per-block transposed by the norm-mul for free); products as two
    DVE/Pool broadcast-AP half-products per c-tile with the reduced axis
    innermost (j local, i global) so all operands have packed 2-byte last
    dims -> DVE 2x mode. Reduction via h-identity (f32r) + 16 plane
    identity matmuls accumulating h+update in PSUM; h copy-back on ACT/DVE.
  - GLU MLP: fp8e4m3 DoubleRow PE matmuls (wv/wg/wo scaled by 64 to dodge
    e4m3 subnormals; descales folded into ACT silu scale, DVE gate stt, and
    the 512*I identity matmul + 1/512 copy-back). Silu on ACT, gates on DVE.
  - rms weights / alphas are folded into the mixer/GLU weights on host.
"""

import numpy as np
import ml_dtypes

import concourse.bacc as bacc
import concourse.bass as bass
import concourse.tile as tile
import concourse.mybir as mybir
from concourse.bass_utils import run_bass_kernel_spmd

B, S, H, GLU, VOC, L, CIN, BLK = 8, 256, 512, 1024, 256, 24, 320, 16
EPS = 1e-5
NT = H // 128  # 4 channel tiles
GT = GLU // 128  # 8 glu tiles

F32 = mybir.dt.float32
F32R = mybir.dt.float32r
BF16 = mybir.dt.bfloat16
FP8 = mybir.dt.float8e4
MULT = mybir.AluOpType.mult
ADD = mybir.AluOpType.add
AF = mybir.ActivationFunctionType
DR = mybir.MatmulPerfMode.DoubleRow

WSC = 64.0  # fp8 weight scale for wv/wg/wo (avoids e4m3 subnormals)
GSC = 8.0  # fp8 gate activation scale

# mixer product halves run on Pool (rest on DVE): (tile, half) pairs
POOL_HALVES = ((0, 1), (1, 1), (2, 1))
CHAIN_ORDER = (0, 1, 2, 3)

# warm-keeper matmul counts (fill PE idle windows to hold the p-state ramp)
WARM = {"rms": 0, "bcast": 0, "mixer": 0, "glu_p": 0, "glu_o": 0}

_PROG_CACHE = {}


def _bview(base, doff, free_dims):
    """View of a 2D sbuf AP with custom (possibly broadcast) free dims."""
    return bass.AP(
        tensor=base.tensor,
        offset=base.offset + doff,
        ap=[list(base.ap[0])] + [list(d) for d in free_dims],
    )


def build_program(n_layers=L, sim_compat=False):
    nc = bacc.Bacc("TRN2")

    x_d = nc.dram_tensor("x", [384, S], F32R, kind="ExternalInput")
    stw_d = nc.dram_tensor("stem_wT", [384, H], F32R, kind="ExternalInput")
    wv_d = nc.dram_tensor("wvT", [n_layers, H, GLU], FP8, kind="ExternalInput")
    wg_d = nc.dram_tensor("wgT", [n_layers, H, GLU], FP8, kind="ExternalInput")
    wo_d = nc.dram_tensor("woT", [n_layers, GLU, H], FP8, kind="ExternalInput")
    wl_d = nc.dram_tensor("wl", [n_layers, H, 256], BF16, kind="ExternalInput")
    wm_d = nc.dram_tensor("wm", [n_layers, H, 256], BF16, kind="ExternalInput")
    hw_d = nc.dram_tensor("headT", [H, VOC], BF16, kind="ExternalInput")
    id_d = nc.dram_tensor("ident", [128, 128], BF16, kind="ExternalInput")
    idr_d = nc.dram_tensor("ident_r", [128, 128], F32R, kind="ExternalInput")
    id5_d = nc.dram_tensor("ident512", [128, 128], F32R, kind="ExternalInput")
    ones_d = nc.dram_tensor("ones_k", [128, 1], F32R, kind="ExternalInput")
    onesr_d = nc.dram_tensor("ones_m", [1, 128], F32R, kind="ExternalInput")
    out_d = nc.dram_tensor("out", [VOC, S], F32, kind="ExternalOutput")

    from contextlib import ExitStack

    with tile.TileContext(nc) as tc, ExitStack() as ctx:
        singles = ctx.enter_context(tc.tile_pool(name="singles", bufs=1))
        wpool = ctx.enter_context(tc.tile_pool(name="wpool", bufs=2))
        hpool = ctx.enter_context(tc.tile_pool(name="hpool", bufs=1))
        npool = ctx.enter_context(tc.tile_pool(name="npool", bufs=2))
        apool = ctx.enter_context(tc.tile_pool(name="apool", bufs=3))
        ppool = ctx.enter_context(tc.tile_pool(name="ppool", bufs=5))
        gpool = ctx.enter_context(tc.tile_pool(name="gpool", bufs=2))
        ps_n = ctx.enter_context(tc.tile_pool(name="ps_n", bufs=1, space="PSUM"))
        ps_m = ctx.enter_context(tc.tile_pool(name="ps_m", bufs=2, space="PSUM"))
        ps_g = ctx.enter_context(tc.tile_pool(name="ps_g", bufs=3, space="PSUM"))
        ps_o = ctx.enter_context(tc.tile_pool(name="ps_o", bufs=2, space="PSUM"))

        # ---- constants / stem operands ----
        ident = singles.tile([128, 128], BF16, tag="ident")
        nc.sync.dma_start(out=ident, in_=id_d[:])
        identr_st = singles.tile([128, 128], F32R, tag="identr_st")
        nc.sync.dma_start(out=identr_st, in_=idr_d[:])
        identr = singles.tile([128, 128], F32R, tag="identr")
        id512_st = singles.tile([128, 128], F32R, tag="id512_st")
        nc.sync.dma_start(out=id512_st, in_=id5_d[:])
        id512 = singles.tile([128, 128], F32R, tag="id512")
        ones_k_st = singles.tile([128, 1], F32R, tag="ones_k_st")
        nc.sync.dma_start(out=ones_k_st, in_=ones_d[:])
        ones_k = singles.tile([128, 1], F32R, tag="ones_k")
        ones_m_st = singles.tile([1, 128], F32R, tag="ones_m_st")
        nc.sync.dma_start(out=ones_m_st, in_=onesr_d[:])
        ones_m = singles.tile([1, 128], F32R, tag="ones_m")
        eps_sb = singles.tile([1, 1], F32, tag="eps")
        nc.vector.memset(eps_sb, float(EPS))
        dmy = singles.tile([1, 1], F32, tag="dmy")

        def preload_table(func, dep=None):
            # dummy op to hoist the ACT table reload off the critical path;
            # dep pins the earliest-start so the scheduler overlaps the load
            nc.scalar.activation(dmy, eps_sb if dep is None else dep, func)

        x_st = singles.tile([128, 3, S], F32R, tag="x_st")
        nc.sync.dma_start(out=x_st, in_=x_d[:].rearrange("(t p) s -> p t s", p=128))
        x_sb = singles.tile([128, 3, S], F32R, tag="x")
        stw_st = singles.tile([128, 3, H], F32R, tag="stw_st")
        nc.sync.dma_start(out=stw_st, in_=stw_d[:].rearrange("(t p) s -> p t s", p=128))
        stw_sb = singles.tile([128, 3, H], F32R, tag="stw")

        # Route fp32r matmul operands through a DVE copy so each matmul's
        # operand has an engine writer (a matmul can carry only one
        # cross-engine wait through walrus codegen). Touch bf16 weight DMAs
        # with ldweights for the same reason.
        with nc.allow_low_precision(reason="fp32r staging copies"):
            nc.vector.tensor_copy(out=ones_k, in_=ones_k_st)
            nc.vector.tensor_copy(out=ones_m, in_=ones_m_st)
            nc.vector.tensor_copy(out=x_sb, in_=x_st)
            nc.vector.tensor_copy(out=stw_sb, in_=stw_st)
            nc.vector.tensor_copy(out=identr, in_=identr_st)
            nc.vector.tensor_copy(out=id512, in_=id512_st)
        nc.tensor.ldweights(ident[:, 0:128])

        # ---- h tiles (resident, f32r so identity matmuls can stream them) ----
        h = [
            hpool.tile([128, S], F32R, tag=f"h{t}", name=f"h{t}") for t in range(NT)
        ]

        # warm-keeper: dummy matmuls into the spare region of the broadcast
        # PSUM bank; they only run when PE is otherwise idle and keep the
        # p-state ramp alive through dependency stalls
        warm_dest = [None]

        def warm(n):
            if warm_dest[0] is None or n == 0:
                return
            for _ in range(n):
                nc.tensor.matmul(
                    warm_dest[0],
                    ident[:, 0:64],
                    ident[:],
                    start=True,
                    stop=True,
                    skip_group_check=True,
                )

        # ---- stem: h = stem_w @ x ----
        for t in range(NT):
            pst = ps_o.tile([128, S], F32, tag="po")
            for kt in range(3):
                nc.tensor.matmul(
                    pst,
                    stw_sb[:, kt, t * 128 : (t + 1) * 128],
                    x_sb[:, kt, :],
                    start=(kt == 0),
                    stop=(kt == 2),
                )
            with nc.allow_low_precision(reason="f32r h tiles"):
                nc.vector.tensor_copy(out=h[t], in_=pst)

        def rms_sbuf(srcs=None, jp=False):
            """Returns SBUF [128, S] f32 broadcast of 1/sqrt(mean(h^2)+eps).

            srcs: optional per-tile (psum_acc, scale) pairs holding
            scale*acc == h_new; squares then read the PSUM accs directly so
            the rms chain starts before the h copy-back lands.
            jp: srcs accs are (j,p)-ordered; h reads must match that order
            (rb then comes back (j,p)-ordered too).
            """
            msrb = ps_n.tile([128, 512], F32, tag="msrb")
            warm(WARM["rms"])
            ms = msrb[0:1, 256:512]
            for t in range(NT):
                sq = apool.tile([128, S], F32R, tag="sq")
                # t>=2 reads the PSUM acc directly (their banks' next writers
                # come after these squares in program order, so no WAR
                # circularity); t<2 reads h after the early copy-back
                hv = h[t][:] if not jp else _bview(h[t][:], 0, [[1, 16], [16, 16]])
                if srcs is not None and t >= 2:
                    src, sc = srcs[t]
                    with nc.allow_low_precision(reason="fp32r squares"):
                        nc.scalar.activation(sq, src, AF.Square, scale=sc)
                elif t < 2:
                    nc.scalar.square(sq, hv)
                else:
                    with nc.allow_low_precision(reason="fp32r squares"):
                        nc.vector.tensor_tensor(out=sq, in0=hv, in1=hv, op=MULT)
                nc.tensor.matmul(
                    ms,
                    ones_k[:, 0:1],
                    sq[:],
                    start=(t == 0),
                    stop=(t == NT - 1),
                )
            stdv = npool.tile([1, S], F32R, tag="stdv")
            with nc.allow_low_precision(reason="fp32r stdv for broadcast matmul"):
                nc.scalar.activation(
                    stdv, ms, AF.Sqrt, bias=eps_sb[0:1, 0:1], scale=1.0 / H
                )
            warm(WARM["bcast"])
            sdb = msrb[:, 0:256]
            nc.tensor.matmul(
                sdb,
                ones_m[0:1, :],
                stdv[:],
                start=True,
                stop=True,
            )
            rb = npool.tile([128, S], F32, tag="rbs")
            nc.vector.reciprocal(rb, sdb)
            return rb

        def mixer(wbase_sb, glob, prod_eng_pool, srcs=None):
            """One mixer sublayer over all 4 c-tiles. Returns PSUM accs.

            local (glob=False): out[c,i,p] = sum_j Wl[c,(p,j)] u[c,(i,j)]
            global (glob=True):  out[c,p,j] = sum_i Wm[c,(p,i)] v[c,(i,j)]
            """
            rb = rms_sbuf(srcs)
            uns = []
            for t in range(NT):
                un = apool.tile([128, S], BF16, tag=f"u{t}")
                if not glob:
                    # u in natural (i,j) order
                    nc.gpsimd.tensor_tensor(out=un, in0=h[t], in1=rb, op=MULT)
                else:
                    # v written per-block transposed: vT[c, 16j+i]
                    nc.gpsimd.tensor_tensor(
                        out=_bview(un[:], 0, [[1, 16], [16, 16]]),
                        in0=_bview(h[t][:], 0, [[16, 16], [1, 16]]),
                        in1=_bview(rb[:], 0, [[16, 16], [1, 16]]),
                        op=MULT,
                    )
                uns.append(un)
            # products split into halves along the innermost (reduced) axis;
            # each half its own tile so identity matmuls start after half A.
            # local: out (i,p,j); in0 u (i,p,j); in1 wl (i,p,j)
            # glob:  out (j,p,i); in0 vT (j,p,i); in1 wm (j,p,i)
            prods = []
            for t in range(NT):
                wbase = wbase_sb[:, t, :]
                halves = []
                for hf in range(2):
                    ph = ppool.tile([128, 2048], BF16, tag=f"prod{hf}")
                    eng = nc.gpsimd if (t, hf) in prod_eng_pool else nc.vector
                    eng.tensor_tensor(
                        out=_bview(ph[:], 0, [[128, 16], [8, 16], [1, 8]]),
                        in0=_bview(uns[t][:], 8 * hf, [[16, 16], [0, 16], [1, 8]]),
                        in1=_bview(wbase, 8 * hf, [[0, 16], [16, 16], [1, 8]]),
                        op=MULT,
                    )
                    halves.append(ph)
                prods.append(halves)
            warm(WARM["mixer"])
            accs = [None] * NT
            # chains ordered so the tile fed by Pool's last product comes
            # last; t3 (fully DVE-fed) is ready before t2's Pool half
            for t in CHAIN_ORDER:
                acc = ps_m.tile([128, S], F32, tag="macc")
                # fold h into the PSUM chain via an f32r identity matmul so
                # the update is a pure PSUM->SBUF copy (GPSIMD can't read
                # PSUM on HW); h streamed in the same (X,p) order as acc
                hr = h[t][:]
                nc.tensor.matmul(
                    acc,
                    identr[:],
                    hr if not glob else _bview(hr, 0, [[1, 16], [16, 16]]),
                    start=True,
                    stop=False,
                )
                for q in range(16):
                    hf, qq = divmod(q, 8)
                    nc.tensor.matmul(
                        acc,
                        ident[:],
                        _bview(prods[t][hf][:], qq, [[128, 16], [8, 16]]),
                        start=False,
                        stop=(q == 15),
                    )
                # h copy-back: (i,p) is natural s' order for local; (j,p)
                # for global, where h[c, 16p+j] is viewed as (j,p)
                hv = h[t][:] if not glob else _bview(h[t][:], 0, [[1, 16], [16, 16]])
                if t < 2:
                    with nc.allow_low_precision(reason="f32r h tiles"):
                        nc.scalar.activation(hv, acc, AF.Identity)
                else:
                    with nc.allow_low_precision(reason="f32r h tiles"):
                        nc.vector.tensor_copy(out=hv, in_=acc)
                accs[t] = acc
            return accs

        last_accs = None  # previous sublayer's (psum_acc, scale) per tile
        for l in range(n_layers):
            wv_sb = wpool.tile([128, NT, GLU], FP8, tag="wv")
            nc.sync.dma_start(
                out=wv_sb, in_=wv_d[l].rearrange("(t p) o -> p t o", p=128)
            )
            wg_sb = wpool.tile([128, NT, GLU], FP8, tag="wg")
            nc.sync.dma_start(
                out=wg_sb, in_=wg_d[l].rearrange("(t p) o -> p t o", p=128)
            )
            wo_sb = wpool.tile([128, GT, H], FP8, tag="wo")
            nc.sync.dma_start(
                out=wo_sb, in_=wo_d[l].rearrange("(t p) c -> p t c", p=128)
            )
            wl_sb = wpool.tile([128, NT, 256], BF16, tag="wl")
            nc.sync.dma_start(
                out=wl_sb, in_=wl_d[l].rearrange("(t p) q -> p t q", p=128)
            )
            wm_sb = wpool.tile([128, NT, 256], BF16, tag="wm")
            nc.sync.dma_start(
                out=wm_sb, in_=wm_d[l].rearrange("(t p) q -> p t q", p=128)
            )
            nc.tensor.ldweights(wv_sb[:, 0, 0:128])
            nc.tensor.ldweights(wg_sb[:, 0, 0:128])
            nc.tensor.ldweights(wo_sb[:, 0, 0:128])

            # ---------- local mixer: out[c,i,p] = sum_j Wl[c,p,j] u[c,i,j]
            local_accs = mixer(
                wl_sb, glob=False, prod_eng_pool=POOL_HALVES, srcs=last_accs
            )
            # ---------- global mixer: out[c,p,j] = sum_i Wm[c,p,i] v[c,i,j]
            global_accs = mixer(
                wm_sb, glob=True, prod_eng_pool=POOL_HALVES,
                srcs=[(a, 1.0) for a in local_accs],
            )

            # ---------- GLU MLP (fp8 DoubleRow matmuls; weights scaled by WSC)
            # global accs are (j,p)-ordered, so rb comes back (j,p)-ordered:
            # rb[16j+p] holds the value for position s=16p+j
            rb = rms_sbuf([(a, 1.0) for a in global_accs], jp=True)
            # wn pairs: [128, 2, S] fp8 per k-tile pair for DoubleRow rhs;
            # written per-position via (p,j) views to undo rb's ordering
            wn = []
            for q in range(NT // 2):
                wp = apool.tile([128, 2, S], FP8, tag=f"wn{q}")
                for r in range(2):
                    nc.gpsimd.tensor_tensor(
                        out=_bview(wp[:, r, :], 0, [[16, 16], [1, 16]]),
                        in0=_bview(h[2 * q + r][:], 0, [[16, 16], [1, 16]]),
                        in1=_bview(rb[:], 0, [[1, 16], [16, 16]]),
                        op=MULT,
                    )
                wn.append(wp)
            warm(WARM["glu_p"])
            gts = []
            for ot in range(GT):
                p1 = ps_g.tile([128, S], F32, tag="pg")
                for q in range(NT // 2):
                    nc.tensor.matmul(
                        p1,
                        wv_sb[:, 2 * q : 2 * q + 2, ot * 128 : (ot + 1) * 128],
                        wn[q][:],
                        start=(q == 0),
                        stop=(q == NT // 2 - 1),
                        perf_mode=DR,
                    )
                s1 = apool.tile([128, S], FP8, tag="s1")
                if sim_compat:
                    # CoreSim has no Silu: emulate with Sigmoid + extra mul
                    sg = apool.tile([128, S], BF16, tag="sg")
                    nc.scalar.activation(sg, p1, AF.Sigmoid, scale=1.0 / WSC)
                    nc.vector.scalar_tensor_tensor(
                        out=s1, in0=p1, scalar=1.0 / WSC, in1=sg,
                        op0=MULT, op1=MULT,
                    )
                else:
                    nc.scalar.activation(s1, p1, AF.Silu, scale=1.0 / WSC)
                p3 = ps_g.tile([128, S], F32, tag="pg")
                for q in range(NT // 2):
                    nc.tensor.matmul(
                        p3,
                        wg_sb[:, 2 * q : 2 * q + 2, ot * 128 : (ot + 1) * 128],
                        wn[q][:],
                        start=(q == 0),
                        stop=(q == NT // 2 - 1),
                        perf_mode=DR,
                    )
                # gt = (p3 / WSC * GSC) * s1, stored fp8 (scaled by GSC)
                qg, rg_ = divmod(ot, 2)
                if rg_ == 0:
                    gp = gpool.tile([128, 2, S], FP8, tag=f"g{qg}")
                    gts.append(gp)
                nc.vector.scalar_tensor_tensor(
                    out=gts[qg][:, rg_, :], in0=p3, scalar=GSC / WSC, in1=s1,
                    op0=MULT, op1=MULT,
                )
            warm(WARM["glu_o"])
            last_accs = []
            for t in range(NT):
                po = ps_o.tile([128, S], F32, tag="po")
                # acc = (WSC*GSC)*h + (WSC*GSC)*update via a scaled identity;
                # h update is then a pure ACT copy with scale 1/(WSC*GSC)
                nc.tensor.matmul(
                    po,
                    id512[:],
                    h[t][:],
                    start=True,
                    stop=False,
                )
                for q in range(GT // 2):
                    nc.tensor.matmul(
                        po,
                        wo_sb[:, 2 * q : 2 * q + 2, t * 128 : (t + 1) * 128],
                        gts[q][:],
                        start=False,
                        stop=(q == GT // 2 - 1),
                        perf_mode=DR,
                    )
                if t < 2:
                    with nc.allow_low_precision(reason="f32r h tiles"):
                        nc.scalar.activation(
                            h[t], po, AF.Identity, scale=1.0 / (WSC * GSC)
                        )
                else:
                    with nc.allow_low_precision(reason="f32r h tiles"):
                        nc.vector.tensor_scalar_mul(
                            out=h[t], in0=po, scalar1=1.0 / (WSC * GSC)
                        )
                last_accs.append((po, 1.0 / (WSC * GSC)))

        # ---------- head ----------
        hw_sb = singles.tile([128, NT, VOC], BF16, tag="hw")
        nc.sync.dma_start(out=hw_sb, in_=hw_d.rearrange("(t p) v -> p t v", p=128))
        nc.tensor.ldweights(hw_sb[:, 0, 0:128])
        rb = rms_sbuf(last_accs)
        nrm = []
        for t in range(NT):
            n_ = apool.tile([128, S], BF16, tag=f"wn{t}")
            nc.vector.tensor_tensor(out=n_, in0=h[t], in1=rb, op=MULT)
            nrm.append(n_)
        for mc in range(VOC // 128):
            po = ps_o.tile([128, S], F32, tag="po")
            for kt in range(NT):
                nc.tensor.matmul(
                    po,
                    hw_sb[:, kt, mc * 128 : (mc + 1) * 128],
                    nrm[kt][:],
                    start=(kt == 0),
                    stop=(kt == NT - 1),
                )
            ot_sb = apool.tile([128, S], F32, tag="osb")
            nc.vector.tensor_copy(out=ot_sb, in_=po)
            nc.sync.dma_start(out=out_d[mc * 128 : (mc + 1) * 128, :], in_=ot_sb)

    nc.compile()
    return nc


def _prep_inputs(inputs, n_layers=L):
    """Host-side weight folding + layout prep. Returns dict of np arrays."""
    f = lambda k: np.asarray(inputs[k], dtype=np.float32)
    x = f("x")
    stem_w = f("stem_w")  # [H, CIN]
    rl, rg, rf = f("rms_local"), f("rms_global"), f("rms_ffn")
    al, ag, am = f("alpha_local"), f("alpha_global"), f("alpha_mlp")
    w_local, w_global = f("w_local"), f("w_global")  # [L, H, BLK, BLK]
    wv, wg, wo = f("wv"), f("wg"), f("wo")
    head_rms, head_w = f("head_rms"), f("head_w")
    hls = np.float32(np.asarray(inputs["head_logit_scale"]))

    bf = ml_dtypes.bfloat16
    nl = n_layers

    # local: fold alpha_local * rms_local[c] into Wl[c,p,j]; layout [c, 16p+j]
    wl_h = (w_local[:nl] * al[:nl, None, None, None] * rl[:nl, :, None, None]).reshape(
        nl, H, 256
    )
    # global: Wg[c,p,i]; layout [c, 16p+i]
    wm_h = (w_global[:nl] * ag[:nl, None, None, None] * rg[:nl, :, None, None]).reshape(
        nl, H, 256
    )
    # GLU: fold rms_ffn into wv/wg columns; alpha_mlp into wo
    wvT = np.ascontiguousarray(
        np.transpose(wv[:nl] * rf[:nl, None, :], (0, 2, 1))
    )  # [L, H, GLU]
    wgT = np.ascontiguousarray(np.transpose(wg[:nl] * rf[:nl, None, :], (0, 2, 1)))
    woT = np.ascontiguousarray(
        np.transpose(wo[:nl] * am[:nl, None, None], (0, 2, 1))
    )  # [L, GLU, H]
    headT = np.ascontiguousarray((head_w * head_rms[None, :] * hls).T)  # [H, VOC]

    stw_pad = np.zeros((384, H), np.float32)
    stw_pad[:CIN] = stem_w.T
    f8 = ml_dtypes.float8_e4m3
    common = {
        "stem_wT": stw_pad,  # [384, H] zero-padded
        "wvT": (wvT * 64.0).astype(f8),
        "wgT": (wgT * 64.0).astype(f8),
        "woT": (woT * 64.0).astype(f8),
        "wl": wl_h.astype(bf),
        "wm": wm_h.astype(bf),
        "headT": headT.astype(bf),
        "ident": np.eye(128, dtype=bf),
        "ident_r": np.eye(128, dtype=np.float32),
        "ident512": (512.0 * np.eye(128)).astype(np.float32),
        "ones_k": np.ones((128, 1), np.float32),
        "ones_m": np.ones((1, 128), np.float32),
    }
    per_core = []
    for b in range(B):
        xp = np.zeros((384, S), np.float32)
        xp[:CIN] = x[b, :, 0, :]
        per_core.append(dict(common, x=xp))
    return per_core


def run(inputs, n_layers=L, trace=False):
    key = n_layers
    if key not in _PROG_CACHE:
        _PROG_CACHE[key] = build_program(n_layers)
    nc = _PROG_CACHE[key]
    in_maps = _prep_inputs(inputs, n_layers)
    res = run_bass_kernel_spmd(nc, in_maps, core_ids=list(range(B)), trace=trace)
    out = np.stack([r["out"] for r in res.results])  # [B, VOC, S]
    return out[:, :, None, :].astype(np.float32), res


def kernel(**inputs):
    out, _ = run(inputs, L, trace=False)
    return out


# revision 84
# speedup vs baseline: 1.4676x; 1.0094x over previous
"""Trainium2 Bass kernel for nn_ByteGridModel (dense_cnn).

Sharding: pure data-parallel over batch B=8 -> 8 cores, one batch item per
core, no collectives. Weights replicated (streamed per layer, double
buffered).

Per-core layout: channels on partitions, h = [H=512 -> 4x128, S=256] f32r
resident in SBUF (f32r so identity matmuls can stream h into PSUM chains).

Per layer (engine assignment tuned against the CoreSim cost model; GPSIMD
never touches PSUM - the HW BIR verifier rejects that):
  - rmsnorm: squares on ACT (read previous sublayer's PSUM accs directly
    where the bank rotation allows, so the chain starts before h lands) ->
    f32r ones-matmul partition reduction -> ACT sqrt -> f32r broadcast
    matmul -> DVE reciprocal into SBUF.
  - per-channel 16x16 mixers: norm-muls on Pool (the global mixer's v is
    written per-block transposed by the norm-mul for free); products as two
    DVE/Pool broadcast-AP half-products per c-tile with the reduced axis
    innermost (j local, i global) so all operands have packed 2-byte last
    dims -> DVE 2x mode. Reduction via h-identity (f32r) + 16 plane
    identity matmuls accumulating h+update in PSUM; h copy-back on ACT/DVE.
  - GLU MLP: fp8e4m3 DoubleRow PE matmuls (wv/wg/wo scaled by 64 to dodge
    e4m3 subnormals; descales folded into ACT silu scale, DVE gate stt, and
    the 512*I identity matmul + 1/512 copy-back). Silu on ACT, gates on DVE.
  - rms weights / alphas are folded into the mixer/GLU weights on host.
"""

import numpy as np
import ml_dtypes

import concourse.bacc as bacc
import concourse.bass as bass
import concourse.tile as tile
import concourse.mybir as mybir
from concourse.bass_utils import run_bass_kernel_spmd

B, S, H, GLU, VOC, L, CIN, BLK = 8, 256, 512, 1024, 256, 24, 320, 16
EPS = 1e-5
NT = H // 128  # 4 channel tiles
GT = GLU // 128  # 8 glu tiles

F32 = mybir.dt.float32
F32R = mybir.dt.float32r
BF16 = mybir.dt.bfloat16
FP8 = mybir.dt.float8e4
MULT = mybir.AluOpType.mult
ADD = mybir.AluOpType.add
AF = mybir.ActivationFunctionType
DR = mybir.MatmulPerfMode.DoubleRow

WSC = 64.0  # fp8 weight scale for wv/wg/wo (avoids e4m3 subnormals)
GSC = 8.0  # fp8 gate activation scale

# mixer product halves run on Pool (rest on DVE): (tile, half) pairs
POOL_HALVES = ((0, 1), (1, 1), (2, 1))
CHAIN_ORDER = (0, 1, 2, 3)
NORM_DVE = (0,)  # mixer norm-mul tiles computed on DVE instead of Pool

# warm-keeper matmul counts (fill PE idle windows to hold the p-state ramp)
WARM = {"rms": 0, "bcast": 0, "mixer": 0, "glu_p": 0, "glu_o": 0}

_PROG_CACHE = {}


def _bview(base, doff, free_dims):
    """View of a 2D sbuf AP with custom (possibly broadcast) free dims."""
    return bass.AP(
        tensor=base.tensor,
        offset=base.offset + doff,
        ap=[list(base.ap[0])] + [list(d) for d in free_dims],
    )


def build_program(n_layers=L, sim_compat=False):
    nc = bacc.Bacc("TRN2")

    x_d = nc.dram_tensor("x", [384, S], F32R, kind="ExternalInput")
    stw_d = nc.dram_tensor("stem_wT", [384, H], F32R, kind="ExternalInput")
    wv_d = nc.dram_tensor("wvT", [n_layers, H, GLU], FP8, kind="ExternalInput")
    wg_d = nc.dram_tensor("wgT", [n_layers, H, GLU], FP8, kind="ExternalInput")
    wo_d = nc.dram_tensor("woT", [n_layers, GLU, H], FP8, kind="ExternalInput")
    wl_d = nc.dram_tensor("wl", [n_layers, H, 256], BF16, kind="ExternalInput")
    wm_d = nc.dram_tensor("wm", [n_layers, H, 256], BF16, kind="ExternalInput")
    hw_d = nc.dram_tensor("headT", [H, VOC], BF16, kind="ExternalInput")
    id_d = nc.dram_tensor("ident", [128, 128], BF16, kind="ExternalInput")
    idr_d = nc.dram_tensor("ident_r", [128, 128], F32R, kind="ExternalInput")
    id5_d = nc.dram_tensor("ident512", [128, 128], F32R, kind="ExternalInput")
    ones_d = nc.dram_tensor("ones_k", [128, 1], F32R, kind="ExternalInput")
    onesr_d = nc.dram_tensor("ones_m", [1, 128], F32R, kind="ExternalInput")
    out_d = nc.dram_tensor("out", [VOC, S], F32, kind="ExternalOutput")

    from contextlib import ExitStack

    with tile.TileContext(nc) as tc, ExitStack() as ctx:
        singles = ctx.enter_context(tc.tile_pool(name="singles", bufs=1))
        wpool = ctx.enter_context(tc.tile_pool(name="wpool", bufs=2))
        hpool = ctx.enter_context(tc.tile_pool(name="hpool", bufs=1))
        npool = ctx.enter_context(tc.tile_pool(name="npool", bufs=2))
        apool = ctx.enter_context(tc.tile_pool(name="apool", bufs=3))
        ppool = ctx.enter_context(tc.tile_pool(name="ppool", bufs=5))
        gpool = ctx.enter_context(tc.tile_pool(name="gpool", bufs=2))
        ps_n = ctx.enter_context(tc.tile_pool(name="ps_n", bufs=1, space="PSUM"))
        ps_m = ctx.enter_context(tc.tile_pool(name="ps_m", bufs=2, space="PSUM"))
        ps_g = ctx.enter_context(tc.tile_pool(name="ps_g", bufs=3, space="PSUM"))
        ps_o = ctx.enter_context(tc.tile_pool(name="ps_o", bufs=2, space="PSUM"))

        # ---- constants / stem operands ----
        ident = singles.tile([128, 128], BF16, tag="ident")
        nc.sync.dma_start(out=ident, in_=id_d[:])
        identr_st = singles.tile([128, 128], F32R, tag="identr_st")
        nc.sync.dma_start(out=identr_st, in_=idr_d[:])
        identr = singles.tile([128, 128], F32R, tag="identr")
        id512_st = singles.tile([128, 128], F32R, tag="id512_st")
        nc.sync.dma_start(out=id512_st, in_=id5_d[:])
        id512 = singles.tile([128, 128], F32R, tag="id512")
        ones_k_st = singles.tile([128, 1], F32R, tag="ones_k_st")
        nc.sync.dma_start(out=ones_k_st, in_=ones_d[:])
        ones_k = singles.tile([128, 1], F32R, tag="ones_k")
        ones_m_st = singles.tile([1, 128], F32R, tag="ones_m_st")
        nc.sync.dma_start(out=ones_m_st, in_=onesr_d[:])
        ones_m = singles.tile([1, 128], F32R, tag="ones_m")
        eps_sb = singles.tile([1, 1], F32, tag="eps")
        nc.vector.memset(eps_sb, float(EPS))
        dmy = singles.tile([1, 1], F32, tag="dmy")

        def preload_table(func, dep=None):
            # dummy op to hoist the ACT table reload off the critical path;
            # dep pins the earliest-start so the scheduler overlaps the load
            nc.scalar.activation(dmy, eps_sb if dep is None else dep, func)

        x_st = singles.tile([128, 3, S], F32R, tag="x_st")
        nc.sync.dma_start(out=x_st, in_=x_d[:].rearrange("(t p) s -> p t s", p=128))
        x_sb = singles.tile([128, 3, S], F32R, tag="x")
        stw_st = singles.tile([128, 3, H], F32R, tag="stw_st")
        nc.sync.dma_start(out=stw_st, in_=stw_d[:].rearrange("(t p) s -> p t s", p=128))
        stw_sb = singles.tile([128, 3, H], F32R, tag="stw")

        # Route fp32r matmul operands through a DVE copy so each matmul's
        # operand has an engine writer (a matmul can carry only one
        # cross-engine wait through walrus codegen). Touch bf16 weight DMAs
        # with ldweights for the same reason.
        with nc.allow_low_precision(reason="fp32r staging copies"):
            nc.vector.tensor_copy(out=ones_k, in_=ones_k_st)
            nc.vector.tensor_copy(out=ones_m, in_=ones_m_st)
            nc.vector.tensor_copy(out=x_sb, in_=x_st)
            nc.vector.tensor_copy(out=stw_sb, in_=stw_st)
            nc.vector.tensor_copy(out=identr, in_=identr_st)
            nc.vector.tensor_copy(out=id512, in_=id512_st)
        nc.tensor.ldweights(ident[:, 0:128])

        # ---- h tiles (resident, f32r so identity matmuls can stream them) ----
        h = [
            hpool.tile([128, S], F32R, tag=f"h{t}", name=f"h{t}") for t in range(NT)
        ]

        # warm-keeper: dummy matmuls into the spare region of the broadcast
        # PSUM bank; they only run when PE is otherwise idle and keep the
        # p-state ramp alive through dependency stalls
        warm_dest = [None]

        def warm(n):
            if warm_dest[0] is None or n == 0:
                return
            for _ in range(n):
                nc.tensor.matmul(
                    warm_dest[0],
                    ident[:, 0:64],
                    ident[:],
                    start=True,
                    stop=True,
                    skip_group_check=True,
                )

        # ---- stem: h = stem_w @ x ----
        for t in range(NT):
            pst = ps_o.tile([128, S], F32, tag="po")
            for kt in range(3):
                nc.tensor.matmul(
                    pst,
                    stw_sb[:, kt, t * 128 : (t + 1) * 128],
                    x_sb[:, kt, :],
                    start=(kt == 0),
                    stop=(kt == 2),
                )
            with nc.allow_low_precision(reason="f32r h tiles"):
                nc.vector.tensor_copy(out=h[t], in_=pst)

        def rms_sbuf(srcs=None, jp=False):
            """Returns SBUF [128, S] f32 broadcast of 1/sqrt(mean(h^2)+eps).

            srcs: optional per-tile (psum_acc, scale) pairs holding
            scale*acc == h_new; squares then read the PSUM accs directly so
            the rms chain starts before the h copy-back lands.
            jp: srcs accs are (j,p)-ordered; h reads must match that order
            (rb then comes back (j,p)-ordered too).
            """
            msrb = ps_n.tile([128, 512], F32, tag="msrb")
            warm(WARM["rms"])
            ms = msrb[0:1, 256:512]
            for t in range(NT):
                sq = apool.tile([128, S], F32R, tag="sq")
                # t>=2 reads the PSUM acc directly (their banks' next writers
                # come after these squares in program order, so no WAR
                # circularity); t<2 reads h after the early copy-back
                hv = h[t][:] if not jp else _bview(h[t][:], 0, [[1, 16], [16, 16]])
                if srcs is not None and t >= 2:
                    src, sc = srcs[t]
                    with nc.allow_low_precision(reason="fp32r squares"):
                        nc.scalar.activation(sq, src, AF.Square, scale=sc)
                elif t < 2:
                    nc.scalar.square(sq, hv)
                else:
                    with nc.allow_low_precision(reason="fp32r squares"):
                        nc.vector.tensor_tensor(out=sq, in0=hv, in1=hv, op=MULT)
                nc.tensor.matmul(
                    ms,
                    ones_k[:, 0:1],
                    sq[:],
                    start=(t == 0),
                    stop=(t == NT - 1),
                )
            stdv = npool.tile([1, S], F32R, tag="stdv")
            with nc.allow_low_precision(reason="fp32r stdv for broadcast matmul"):
                nc.scalar.activation(
                    stdv, ms, AF.Sqrt, bias=eps_sb[0:1, 0:1], scale=1.0 / H
                )
            warm(WARM["bcast"])
            sdb = msrb[:, 0:256]
            nc.tensor.matmul(
                sdb,
                ones_m[0:1, :],
                stdv[:],
                start=True,
                stop=True,
            )
            rb = npool.tile([128, S], F32, tag="rbs")
            nc.vector.reciprocal(rb, sdb)
            return rb

        def mixer(wbase_sb, glob, prod_eng_pool, srcs=None):
            """One mixer sublayer over all 4 c-tiles. Returns PSUM accs.

            local (glob=False): out[c,i,p] = sum_j Wl[c,(p,j)] u[c,(i,j)]
            global (glob=True):  out[c,p,j] = sum_i Wm[c,(p,i)] v[c,(i,j)]
            """
            rb = rms_sbuf(srcs)
            uns = []
            for t in range(NT):
                un = apool.tile([128, S], BF16, tag=f"u{t}")
                # t0's norm-mul on DVE so Pool's product stream isn't queued
                # behind it (DVE's first product waits on it anyway)
                eng_n = nc.vector if t in NORM_DVE else nc.gpsimd
                if not glob:
                    # u in natural (i,j) order
                    eng_n.tensor_tensor(out=un, in0=h[t], in1=rb, op=MULT)
                else:
                    # v written per-block transposed: vT[c, 16j+i]
                    eng_n.tensor_tensor(
                        out=_bview(un[:], 0, [[1, 16], [16, 16]]),
                        in0=_bview(h[t][:], 0, [[16, 16], [1, 16]]),
                        in1=_bview(rb[:], 0, [[16, 16], [1, 16]]),
                        op=MULT,
                    )
                uns.append(un)
            # products split into halves along the innermost (reduced) axis;
            # each half its own tile so identity matmuls start after half A.
            # local: out (i,p,j); in0 u (i,p,j); in1 wl (i,p,j)
            # glob:  out (j,p,i); in0 vT (j,p,i); in1 wm (j,p,i)
            prods = []
            for t in range(NT):
                wbase = wbase_sb[:, t, :]
                halves = []
                for hf in range(2):
                    ph = ppool.tile([128, 2048], BF16, tag=f"prod{hf}")
                    eng = nc.gpsimd if (t, hf) in prod_eng_pool else nc.vector
                    eng.tensor_tensor(
                        out=_bview(ph[:], 0, [[128, 16], [8, 16], [1, 8]]),
                        in0=_bview(uns[t][:], 8 * hf, [[16, 16], [0, 16], [1, 8]]),
                        in1=_bview(wbase, 8 * hf, [[0, 16], [16, 16], [1, 8]]),
                        op=MULT,
                    )
                    halves.append(ph)
                prods.append(halves)
            warm(WARM["mixer"])
            accs = [None] * NT
            # chains ordered so the tile fed by Pool's last product comes
            # last; t3 (fully DVE-fed) is ready before t2's Pool half
            for t in CHAIN_ORDER:
                acc = ps_m.tile([128, S], F32, tag="macc")
                # fold h into the PSUM chain via an f32r identity matmul so
                # the update is a pure PSUM->SBUF copy (GPSIMD can't read
                # PSUM on HW); h streamed in the same (X,p) order as acc
                hr = h[t][:]
                nc.tensor.matmul(
                    acc,
                    identr[:],
                    hr if not glob else _bview(hr, 0, [[1, 16], [16, 16]]),
                    start=True,
                    stop=False,
                )
                for q in range(16):
                    hf, qq = divmod(q, 8)
                    nc.tensor.matmul(
                        acc,
                        ident[:],
                        _bview(prods[t][hf][:], qq, [[128, 16], [8, 16]]),
                        start=False,
                        stop=(q == 15),
                    )
                # h copy-back: (i,p) is natural s' order for local; (j,p)
                # for global, where h[c, 16p+j] is viewed as (j,p)
                hv = h[t][:] if not glob else _bview(h[t][:], 0, [[1, 16], [16, 16]])
                if t < 2:
                    with nc.allow_low_precision(reason="f32r h tiles"):
                        nc.scalar.activation(hv, acc, AF.Identity)
                else:
                    with nc.allow_low_precision(reason="f32r h tiles"):
                        nc.vector.tensor_copy(out=hv, in_=acc)
                accs[t] = acc
            return accs

        last_accs = None  # previous sublayer's (psum_acc, scale) per tile
        for l in range(n_layers):
            wv_sb = wpool.tile([128, NT, GLU], FP8, tag="wv")
            nc.sync.dma_start(
                out=wv_sb, in_=wv_d[l].rearrange("(t p) o -> p t o", p=128)
            )
            wg_sb = wpool.tile([128, NT, GLU], FP8, tag="wg")
            nc.sync.dma_start(
                out=wg_sb, in_=wg_d[l].rearrange("(t p) o -> p t o", p=128)
            )
            wo_sb = wpool.tile([128, GT, H], FP8, tag="wo")
            nc.sync.dma_start(
                out=wo_sb, in_=wo_d[l].rearrange("(t p) c -> p t c", p=128)
            )
            wl_sb = wpool.tile([128, NT, 256], BF16, tag="wl")
            nc.sync.dma_start(
                out=wl_sb, in_=wl_d[l].rearrange("(t p) q -> p t q", p=128)
            )
            wm_sb = wpool.tile([128, NT, 256], BF16, tag="wm")
            nc.sync.dma_start(
                out=wm_sb, in_=wm_d[l].rearrange("(t p) q -> p t q", p=128)
            )
            nc.tensor.ldweights(wv_sb[:, 0, 0:128])
            nc.tensor.ldweights(wg_sb[:, 0, 0:128])
            nc.tensor.ldweights(wo_sb[:, 0, 0:128])

            # ---------- local mixer: out[c,i,p] = sum_j Wl[c,p,j] u[c,i,j]
            local_accs = mixer(
                wl_sb, glob=False, prod_eng_pool=POOL_HALVES, srcs=last_accs
            )
            # ---------- global mixer: out[c,p,j] = sum_i Wm[c,p,i] v[c,i,j]
            global_accs = mixer(
                wm_sb, glob=True, prod_eng_pool=POOL_HALVES,
                srcs=[(a, 1.0) for a in local_accs],
            )

            # ---------- GLU MLP (fp8 DoubleRow matmuls; weights scaled by WSC)
            # global accs are (j,p)-ordered, so rb comes back (j,p)-ordered:
            # rb[16j+p] holds the value for position s=16p+j
            rb = rms_sbuf([(a, 1.0) for a in global_accs], jp=True)
            # wn pairs: [128, 2, S] fp8 per k-tile pair for DoubleRow rhs;
            # written per-position via (p,j) views to undo rb's ordering
            wn = []
            for q in range(NT // 2):
                wp = apool.tile([128, 2, S], FP8, tag=f"wn{q}")
                for r in range(2):
                    nc.gpsimd.tensor_tensor(
                        out=_bview(wp[:, r, :], 0, [[16, 16], [1, 16]]),
                        in0=_bview(h[2 * q + r][:], 0, [[16, 16], [1, 16]]),
                        in1=_bview(rb[:], 0, [[1, 16], [16, 16]]),
                        op=MULT,
                    )
                wn.append(wp)
            warm(WARM["glu_p"])
            gts = []
            for ot in range(GT):
                p1 = ps_g.tile([128, S], F32, tag="pg")
                for q in range(NT // 2):
                    nc.tensor.matmul(
                        p1,
                        wv_sb[:, 2 * q : 2 * q + 2, ot * 128 : (ot + 1) * 128],
                        wn[q][:],
                        start=(q == 0),
                        stop=(q == NT // 2 - 1),
                        perf_mode=DR,
                    )
                s1 = apool.tile([128, S], FP8, tag="s1")
                if sim_compat:
                    # CoreSim has no Silu: emulate with Sigmoid + extra mul
                    sg = apool.tile([128, S], BF16, tag="sg")
                    nc.scalar.activation(sg, p1, AF.Sigmoid, scale=1.0 / WSC)
                    nc.vector.scalar_tensor_tensor(
                        out=s1, in0=p1, scalar=1.0 / WSC, in1=sg,
                        op0=MULT, op1=MULT,
                    )
                else:
                    nc.scalar.activation(s1, p1, AF.Silu, scale=1.0 / WSC)
                p3 = ps_g.tile([128, S], F32, tag="pg")
                for q in range(NT // 2):
                    nc.tensor.matmul(
                        p3,
                        wg_sb[:, 2 * q : 2 * q + 2, ot * 128 : (ot + 1) * 128],
                        wn[q][:],
                        start=(q == 0),
                        stop=(q == NT // 2 - 1),
                        perf_mode=DR,
                    )
                # gt = (p3 / WSC * GSC) * s1, stored fp8 (scaled by GSC)
                qg, rg_ = divmod(ot, 2)
                if rg_ == 0:
                    gp = gpool.tile([128, 2, S], FP8, tag=f"g{qg}")
                    gts.append(gp)
                nc.vector.scalar_tensor_tensor(
                    out=gts[qg][:, rg_, :], in0=p3, scalar=GSC / WSC, in1=s1,
                    op0=MULT, op1=MULT,
                )
            warm(WARM["glu_o"])
            last_accs = []
            for t in range(NT):
                po = ps_o.tile([128, S], F32, tag="po")
                # acc = (WSC*GSC)*h + (WSC*GSC)*update via a scaled identity;
                # h update is then a pure ACT copy with scale 1/(WSC*GSC)
                nc.tensor.matmul(
                    po,
                    id512[:],
                    h[t][:],
                    start=True,
                    stop=False,
                )
                for q in range(GT // 2):
                    nc.tensor.matmul(
                        po,
                        wo_sb[:, 2 * q : 2 * q + 2, t * 128 : (t + 1) * 128],
                        gts[q][:],
                        start=False,
                        stop=(q == GT // 2 - 1),
                        perf_mode=DR,
                    )
                if t < 2:
                    with nc.allow_low_precision(reason="f32r h tiles"):
                        nc.scalar.activation(
                            h[t], po, AF.Identity, scale=1.0 / (WSC * GSC)
                        )
                else:
                    with nc.allow_low_precision(reason="f32r h tiles"):
                        nc.vector.tensor_scalar_mul(
                            out=h[t], in0=po, scalar1=1.0 / (WSC * GSC)
                        )
                last_accs.append((po, 1.0 / (WSC * GSC)))

        # ---------- head ----------
        hw_sb = singles.tile([128, NT, VOC], BF16, tag="hw")
        nc.sync.dma_start(out=hw_sb, in_=hw_d.rearrange("(t p) v -> p t v", p=128))
        nc.tensor.ldweights(hw_sb[:, 0, 0:128])
        rb = rms_sbuf(last_accs)
        nrm = []
        for t in range(NT):
            n_ = apool.tile([128, S], BF16, tag=f"wn{t}")
            nc.vector.tensor_tensor(out=n_, in0=h[t], in1=rb, op=MULT)
            nrm.append(n_)
        for mc in range(VOC // 128):
            po = ps_o.tile([128, S], F32, tag="po")
            for kt in range(NT):
                nc.tensor.matmul(
                    po,
                    hw_sb[:, kt, mc * 128 : (mc + 1) * 128],
                    nrm[kt][:],
                    start=(kt == 0),
                    stop=(kt == NT - 1),
                )
            ot_sb = apool.tile([128, S], F32, tag="osb")
            nc.vector.tensor_copy(out=ot_sb, in_=po)
            nc.sync.dma_start(out=out_d[mc * 128 : (mc + 1) * 128, :], in_=ot_sb)

    nc.compile()
    return nc


def _prep_inputs(inputs, n_layers=L):
    """Host-side weight folding + layout prep. Returns dict of np arrays."""
    f = lambda k: np.asarray(inputs[k], dtype=np.float32)
    x = f("x")
    stem_w = f("stem_w")  # [H, CIN]
    rl, rg, rf = f("rms_local"), f("rms_global"), f("rms_ffn")
    al, ag, am = f("alpha_local"), f("alpha_global"), f("alpha_mlp")
    w_local, w_global = f("w_local"), f("w_global")  # [L, H, BLK, BLK]
    wv, wg, wo = f("wv"), f("wg"), f("wo")
    head_rms, head_w = f("head_rms"), f("head_w")
    hls = np.float32(np.asarray(inputs["head_logit_scale"]))

    bf = ml_dtypes.bfloat16
    nl = n_layers

    # local: fold alpha_local * rms_local[c] into Wl[c,p,j]; layout [c, 16p+j]
    wl_h = (w_local[:nl] * al[:nl, None, None, None] * rl[:nl, :, None, None]).reshape(
        nl, H, 256
    )
    # global: Wg[c,p,i]; layout [c, 16p+i]
    wm_h = (w_global[:nl] * ag[:nl, None, None, None] * rg[:nl, :, None, None]).reshape(
        nl, H, 256
    )
    # GLU: fold rms_ffn into wv/wg columns; alpha_mlp into wo
    wvT = np.ascontiguousarray(
        np.transpose(wv[:nl] * rf[:nl, None, :], (0, 2, 1))
    )  # [L, H, GLU]
    wgT = np.ascontiguousarray(np.transpose(wg[:nl] * rf[:nl, None, :], (0, 2, 1)))
    woT = np.ascontiguousarray(
        np.transpose(wo[:nl] * am[:nl, None, None], (0, 2, 1))
    )  # [L, GLU, H]
    headT = np.ascontiguousarray((head_w * head_rms[None, :] * hls).T)  # [H, VOC]

    stw_pad = np.zeros((384, H), np.float32)
    stw_pad[:CIN] = stem_w.T
    f8 = ml_dtypes.float8_e4m3
    common = {
        "stem_wT": stw_pad,  # [384, H] zero-padded
        "wvT": (wvT * 64.0).astype(f8),
        "wgT": (wgT * 64.0).astype(f8),
        "woT": (woT * 64.0).astype(f8),
        "wl": wl_h.astype(bf),
        "wm": wm_h.astype(bf),
        "headT": headT.astype(bf),
        "ident": np.eye(128, dtype=bf),
        "ident_r": np.eye(128, dtype=np.float32),
        "ident512": (512.0 * np.eye(128)).astype(np.float32),
        "ones_k": np.ones((128, 1), np.float32),
        "ones_m": np.ones((1, 128), np.float32),
    }
    per_core = []
    for b in range(B):
        xp = np.zeros((384, S), np.float32)
        xp[:CIN] = x[b, :, 0, :]
        per_core.append(dict(common, x=xp))
    return per_core


def run(inputs, n_layers=L, trace=False):
    key = n_layers
    if key not in _PROG_CACHE:
        _PROG_CACHE[key] = build_program(n_layers)
    nc = _PROG_CACHE[key]
    in_maps = _prep_inputs(inputs, n_layers)
    res = run_bass_kernel_spmd(nc, in_maps, core_ids=list(range(B)), trace=trace)
    out = np.stack([r["out"] for r in res.results])  # [B, VOC, S]
    return out[:, :, None, :].astype(np.float32), res


def kernel(**inputs):
    out, _ = run(inputs, L, trace=False)
    return out


# revision 85
# speedup vs baseline: 1.5286x; 1.0416x over previous
"""Trainium2 Bass kernel for nn_ByteGridModel (dense_cnn).

Sharding: pure data-parallel over batch B=8 -> 8 cores, one batch item per
core, no collectives. Weights replicated (streamed per layer, double
buffered).

Per-core layout: channels on partitions, h = [H=512 -> 4x128, S=256] f32r
resident in SBUF (f32r so identity matmuls can stream h into PSUM chains).

Per layer (engine assignment tuned against the CoreSim cost model; GPSIMD
never touches PSUM - the HW BIR verifier rejects that):
  - rmsnorm: squares on ACT (read previous sublayer's PSUM accs directly
    where the bank rotation allows, so the chain starts before h lands) ->
    f32r ones-matmul partition reduction -> ACT sqrt -> f32r broadcast
    matmul -> DVE reciprocal into SBUF.
  - per-channel 16x16 mixers: norm-muls on Pool (the global mixer's v is
    written per-block transposed by the norm-mul for free); products as two
    DVE/Pool broadcast-AP half-products per c-tile with the reduced axis
    innermost (j local, i global) so all operands have packed 2-byte last
    dims -> DVE 2x mode. Reduction via h-identity (f32r) + 16 plane
    identity matmuls accumulating h+update in PSUM; h copy-back on ACT/DVE.
  - GLU MLP: fp8e4m3 DoubleRow PE matmuls (wv/wg/wo scaled by 64 to dodge
    e4m3 subnormals; descales folded into ACT silu scale, DVE gate stt, and
    the 512*I identity matmul + 1/512 copy-back). Silu on ACT, gates on DVE.
  - rms weights / alphas are folded into the mixer/GLU weights on host.
"""

import numpy as np
import ml_dtypes

import concourse.bacc as bacc
import concourse.bass as bass
import concourse.tile as tile
import concourse.mybir as mybir
from concourse.bass_utils import run_bass_kernel_spmd

B, S, H, GLU, VOC, L, CIN, BLK = 8, 256, 512, 1024, 256, 24, 320, 16
EPS = 1e-5
NT = H // 128  # 4 channel tiles
GT = GLU // 128  # 8 glu tiles

F32 = mybir.dt.float32
F32R = mybir.dt.float32r
BF16 = mybir.dt.bfloat16
FP8 = mybir.dt.float8e4
MULT = mybir.AluOpType.mult
ADD = mybir.AluOpType.add
AF = mybir.ActivationFunctionType
DR = mybir.MatmulPerfMode.DoubleRow

WSC = 64.0  # fp8 weight scale for wv/wg/wo (avoids e4m3 subnormals)
GSC = 8.0  # fp8 gate activation scale

# mixer product halves run on Pool (rest on DVE): (tile, half) pairs
POOL_HALVES = ((0, 1), (1, 1), (2, 1))
CHAIN_ORDER = (0, 1, 2, 3)
NORM_DVE = (0,)  # mixer norm-mul tiles computed on DVE instead of Pool

# warm-keeper matmul counts (fill PE idle windows to hold the p-state ramp)
WARM = {"rms": 0, "bcast": 0, "mixer": 0, "glu_p": 0, "glu_o": 0}

_PROG_CACHE = {}


def _bview(base, doff, free_dims):
    """View of a 2D sbuf AP with custom (possibly broadcast) free dims."""
    return bass.AP(
        tensor=base.tensor,
        offset=base.offset + doff,
        ap=[list(base.ap[0])] + [list(d) for d in free_dims],
    )


def build_program(n_layers=L, sim_compat=False):
    nc = bacc.Bacc("TRN2")

    x_d = nc.dram_tensor("x", [384, S], F32R, kind="ExternalInput")
    stw_d = nc.dram_tensor("stem_wT", [384, H], F32R, kind="ExternalInput")
    wv_d = nc.dram_tensor("wvT", [n_layers, H, GLU], FP8, kind="ExternalInput")
    wg_d = nc.dram_tensor("wgT", [n_layers, H, GLU], FP8, kind="ExternalInput")
    wo_d = nc.dram_tensor("woT", [n_layers, GLU, H], FP8, kind="ExternalInput")
    wl_d = nc.dram_tensor("wl", [n_layers, H, 256], BF16, kind="ExternalInput")
    wm_d = nc.dram_tensor("wm", [n_layers, H, 256], BF16, kind="ExternalInput")
    hw_d = nc.dram_tensor("headT", [H, VOC], BF16, kind="ExternalInput")
    id_d = nc.dram_tensor("ident", [128, 128], BF16, kind="ExternalInput")
    idr_d = nc.dram_tensor("ident_r", [128, 128], F32R, kind="ExternalInput")
    id5_d = nc.dram_tensor("ident512", [128, 128], F32R, kind="ExternalInput")
    ones_d = nc.dram_tensor("ones_k", [128, 1], F32R, kind="ExternalInput")
    onesr_d = nc.dram_tensor("ones_m", [1, 128], F32R, kind="ExternalInput")
    out_d = nc.dram_tensor("out", [VOC, S], F32, kind="ExternalOutput")

    from contextlib import ExitStack

    with tile.TileContext(nc) as tc, ExitStack() as ctx:
        singles = ctx.enter_context(tc.tile_pool(name="singles", bufs=1))
        wpool = ctx.enter_context(tc.tile_pool(name="wpool", bufs=2))
        hpool = ctx.enter_context(tc.tile_pool(name="hpool", bufs=1))
        npool = ctx.enter_context(tc.tile_pool(name="npool", bufs=2))
        apool = ctx.enter_context(tc.tile_pool(name="apool", bufs=3))
        ppool = ctx.enter_context(tc.tile_pool(name="ppool", bufs=5))
        gpool = ctx.enter_context(tc.tile_pool(name="gpool", bufs=2))
        ps_n = ctx.enter_context(tc.tile_pool(name="ps_n", bufs=1, space="PSUM"))
        ps_m = ctx.enter_context(tc.tile_pool(name="ps_m", bufs=2, space="PSUM"))
        ps_g = ctx.enter_context(tc.tile_pool(name="ps_g", bufs=3, space="PSUM"))
        ps_o = ctx.enter_context(tc.tile_pool(name="ps_o", bufs=2, space="PSUM"))

        # ---- constants / stem operands ----
        ident = singles.tile([128, 128], BF16, tag="ident")
        nc.sync.dma_start(out=ident, in_=id_d[:])
        identr_st = singles.tile([128, 128], F32R, tag="identr_st")
        nc.sync.dma_start(out=identr_st, in_=idr_d[:])
        identr = singles.tile([128, 128], F32R, tag="identr")
        id512_st = singles.tile([128, 128], F32R, tag="id512_st")
        nc.sync.dma_start(out=id512_st, in_=id5_d[:])
        id512 = singles.tile([128, 128], F32R, tag="id512")
        ones_k_st = singles.tile([128, 1], F32R, tag="ones_k_st")
        nc.sync.dma_start(out=ones_k_st, in_=ones_d[:])
        ones_k = singles.tile([128, 1], F32R, tag="ones_k")
        ones_m_st = singles.tile([1, 128], F32R, tag="ones_m_st")
        nc.sync.dma_start(out=ones_m_st, in_=onesr_d[:])
        ones_m = singles.tile([1, 128], F32R, tag="ones_m")
        eps_sb = singles.tile([1, 1], F32, tag="eps")
        nc.vector.memset(eps_sb, float(EPS))
        dmy = singles.tile([1, 1], F32, tag="dmy")

        def preload_table(func, dep=None):
            # dummy op to hoist the ACT table reload off the critical path;
            # dep pins the earliest-start so the scheduler overlaps the load
            nc.scalar.activation(dmy, eps_sb if dep is None else dep, func)

        x_st = singles.tile([128, 3, S], F32R, tag="x_st")
        nc.sync.dma_start(out=x_st, in_=x_d[:].rearrange("(t p) s -> p t s", p=128))
        x_sb = singles.tile([128, 3, S], F32R, tag="x")
        stw_st = singles.tile([128, 3, H], F32R, tag="stw_st")
        nc.sync.dma_start(out=stw_st, in_=stw_d[:].rearrange("(t p) s -> p t s", p=128))
        stw_sb = singles.tile([128, 3, H], F32R, tag="stw")

        # Route fp32r matmul operands through a DVE copy so each matmul's
        # operand has an engine writer (a matmul can carry only one
        # cross-engine wait through walrus codegen). Touch bf16 weight DMAs
        # with ldweights for the same reason.
        with nc.allow_low_precision(reason="fp32r staging copies"):
            nc.vector.tensor_copy(out=ones_k, in_=ones_k_st)
            nc.vector.tensor_copy(out=ones_m, in_=ones_m_st)
            nc.vector.tensor_copy(out=x_sb, in_=x_st)
            nc.vector.tensor_copy(out=stw_sb, in_=stw_st)
            nc.vector.tensor_copy(out=identr, in_=identr_st)
            nc.vector.tensor_copy(out=id512, in_=id512_st)
        nc.tensor.ldweights(ident[:, 0:128])

        # ---- h tiles (resident, f32r so identity matmuls can stream them) ----
        h = [
            hpool.tile([128, S], F32R, tag=f"h{t}", name=f"h{t}") for t in range(NT)
        ]

        # warm-keeper: dummy matmuls into the spare region of the broadcast
        # PSUM bank; they only run when PE is otherwise idle and keep the
        # p-state ramp alive through dependency stalls
        warm_dest = [None]

        def warm(n):
            if warm_dest[0] is None or n == 0:
                return
            for _ in range(n):
                nc.tensor.matmul(
                    warm_dest[0],
                    ident[:, 0:64],
                    ident[:],
                    start=True,
                    stop=True,
                    skip_group_check=True,
                )

        # ---- stem: h = stem_w @ x ----
        for t in range(NT):
            pst = ps_o.tile([128, S], F32, tag="po")
            for kt in range(3):
                nc.tensor.matmul(
                    pst,
                    stw_sb[:, kt, t * 128 : (t + 1) * 128],
                    x_sb[:, kt, :],
                    start=(kt == 0),
                    stop=(kt == 2),
                )
            with nc.allow_low_precision(reason="f32r h tiles"):
                nc.vector.tensor_copy(out=h[t], in_=pst)

        def rms_sbuf(srcs=None, jp=False):
            """Returns SBUF [128, S] f32 broadcast of 1/sqrt(mean(h^2)+eps).

            srcs: optional per-tile (psum_acc, scale) pairs holding
            scale*acc == h_new; squares then read the PSUM accs directly so
            the rms chain starts before the h copy-back lands.
            jp: srcs accs are (j,p)-ordered; h reads must match that order
            (rb then comes back (j,p)-ordered too).
            """
            msrb = ps_n.tile([128, 512], F32, tag="msrb")
            warm(WARM["rms"])
            ms = msrb[0:1, 256:512]
            for t in range(NT):
                sq = apool.tile([128, S], F32R, tag="sq")
                # t>=2 reads the PSUM acc directly (their banks' next writers
                # come after these squares in program order, so no WAR
                # circularity); t<2 reads h after the early copy-back
                hv = h[t][:] if not jp else _bview(h[t][:], 0, [[1, 16], [16, 16]])
                if srcs is not None and t >= 2:
                    src, sc = srcs[t]
                    with nc.allow_low_precision(reason="fp32r squares"):
                        nc.scalar.activation(sq, src, AF.Square, scale=sc)
                elif t < 2:
                    nc.scalar.square(sq, hv)
                else:
                    with nc.allow_low_precision(reason="fp32r squares"):
                        nc.vector.tensor_tensor(out=sq, in0=hv, in1=hv, op=MULT)
                nc.tensor.matmul(
                    ms,
                    ones_k[:, 0:1],
                    sq[:],
                    start=(t == 0),
                    stop=(t == NT - 1),
                )
            stdv = npool.tile([1, S], F32R, tag="stdv")
            with nc.allow_low_precision(reason="fp32r stdv for broadcast matmul"):
                nc.scalar.activation(
                    stdv, ms, AF.Sqrt, bias=eps_sb[0:1, 0:1], scale=1.0 / H
                )
            warm(WARM["bcast"])
            sdb = msrb[:, 0:256]
            nc.tensor.matmul(
                sdb,
                ones_m[0:1, :],
                stdv[:],
                start=True,
                stop=True,
            )
            rb = npool.tile([128, S], F32, tag="rbs")
            nc.vector.reciprocal(rb, sdb)
            return rb

        def mixer(wbase_sb, glob, prod_eng_pool, srcs=None):
            """One mixer sublayer over all 4 c-tiles. Returns PSUM accs.

            local (glob=False): out[c,i,p] = sum_j Wl[c,(p,j)] u[c,(i,j)]
            global (glob=True):  out[c,p,j] = sum_i Wm[c,(p,i)] v[c,(i,j)]
            """
            rb = rms_sbuf(srcs)
            uns = []
            for t in range(NT):
                un = apool.tile([128, S], BF16, tag=f"u{t}")
                # t0's norm-mul on DVE so Pool's product stream isn't queued
                # behind it (DVE's first product waits on it anyway)
                eng_n = nc.vector if t in NORM_DVE else nc.gpsimd
                if not glob:
                    # u in natural (i,j) order
                    eng_n.tensor_tensor(out=un, in0=h[t], in1=rb, op=MULT)
                else:
                    # v written per-block transposed: vT[c, 16j+i]
                    eng_n.tensor_tensor(
                        out=_bview(un[:], 0, [[1, 16], [16, 16]]),
                        in0=_bview(h[t][:], 0, [[16, 16], [1, 16]]),
                        in1=_bview(rb[:], 0, [[16, 16], [1, 16]]),
                        op=MULT,
                    )
                uns.append(un)
            # products split into halves along the innermost (reduced) axis;
            # each half its own tile so identity matmuls start after half A.
            # local: out (i,p,j); in0 u (i,p,j); in1 wl (i,p,j)
            # glob:  out (j,p,i); in0 vT (j,p,i); in1 wm (j,p,i)
            prods = []
            for t in range(NT):
                wbase = wbase_sb[:, t, :]
                halves = []
                for hf in range(2):
                    ph = ppool.tile([128, 2048], BF16, tag=f"prod{hf}")
                    eng = nc.gpsimd if (t, hf) in prod_eng_pool else nc.vector
                    eng.tensor_tensor(
                        out=_bview(ph[:], 0, [[128, 16], [8, 16], [1, 8]]),
                        in0=_bview(uns[t][:], 8 * hf, [[16, 16], [0, 16], [1, 8]]),
                        in1=_bview(wbase, 8 * hf, [[0, 16], [16, 16], [1, 8]]),
                        op=MULT,
                    )
                    halves.append(ph)
                prods.append(halves)
            warm(WARM["mixer"])
            accs = [None] * NT
            # chains ordered so the tile fed by Pool's last product comes
            # last; t3 (fully DVE-fed) is ready before t2's Pool half
            for t in CHAIN_ORDER:
                acc = ps_m.tile([128, S], F32, tag="macc")
                # fold h into the PSUM chain via an f32r identity matmul so
                # the update is a pure PSUM->SBUF copy (GPSIMD can't read
                # PSUM on HW); h streamed in the same (X,p) order as acc
                hr = h[t][:]
                nc.tensor.matmul(
                    acc,
                    identr[:],
                    hr if not glob else _bview(hr, 0, [[1, 16], [16, 16]]),
                    start=True,
                    stop=False,
                )
                for q in range(16):
                    hf, qq = divmod(q, 8)
                    nc.tensor.matmul(
                        acc,
                        ident[:],
                        _bview(prods[t][hf][:], qq, [[128, 16], [8, 16]]),
                        start=False,
                        stop=(q == 15),
                    )
                # h copy-back: (i,p) is natural s' order for local; (j,p)
                # for global, where h[c, 16p+j] is viewed as (j,p)
                hv = h[t][:] if not glob else _bview(h[t][:], 0, [[1, 16], [16, 16]])
                if t < 2:
                    with nc.allow_low_precision(reason="f32r h tiles"):
                        nc.scalar.activation(hv, acc, AF.Identity)
                else:
                    with nc.allow_low_precision(reason="f32r h tiles"):
                        nc.vector.tensor_copy(out=hv, in_=acc)
                accs[t] = acc
            return accs

        last_accs = None  # previous sublayer's (psum_acc, scale) per tile
        for l in range(n_layers):
            wv_sb = wpool.tile([128, NT, GLU], FP8, tag="wv")
            nc.sync.dma_start(
                out=wv_sb, in_=wv_d[l].rearrange("(t p) o -> p t o", p=128)
            )
            wg_sb = wpool.tile([128, NT, GLU], FP8, tag="wg")
            nc.sync.dma_start(
                out=wg_sb, in_=wg_d[l].rearrange("(t p) o -> p t o", p=128)
            )
            wo_sb = wpool.tile([128, GT, H], FP8, tag="wo")
            nc.sync.dma_start(
                out=wo_sb, in_=wo_d[l].rearrange("(t p) c -> p t c", p=128)
            )
            wl_sb = wpool.tile([128, NT, 256], BF16, tag="wl")
            nc.sync.dma_start(
                out=wl_sb, in_=wl_d[l].rearrange("(t p) q -> p t q", p=128)
            )
            wm_sb = wpool.tile([128, NT, 256], BF16, tag="wm")
            nc.sync.dma_start(
                out=wm_sb, in_=wm_d[l].rearrange("(t p) q -> p t q", p=128)
            )
            nc.tensor.ldweights(wv_sb[:, 0, 0:128])
            nc.tensor.ldweights(wg_sb[:, 0, 0:128])
            nc.tensor.ldweights(wo_sb[:, 0, 0:128])

            # ---------- local mixer: out[c,i,p] = sum_j Wl[c,p,j] u[c,i,j]
            local_accs = mixer(
                wl_sb, glob=False, prod_eng_pool=POOL_HALVES, srcs=last_accs
            )
            # ---------- global mixer: out[c,p,j] = sum_i Wm[c,p,i] v[c,i,j]
            global_accs = mixer(
                wm_sb, glob=True, prod_eng_pool=POOL_HALVES,
                srcs=[(a, 1.0) for a in local_accs],
            )

            # ---------- GLU MLP (fp8 DoubleRow matmuls; weights scaled by WSC)
            # global accs are (j,p)-ordered, so rb comes back (j,p)-ordered:
            # rb[16j+p] holds the value for position s=16p+j
            rb = rms_sbuf([(a, 1.0) for a in global_accs], jp=True)
            # wn pairs: [128, 2, S] fp8 per k-tile pair for DoubleRow rhs;
            # written per-position via (p,j) views to undo rb's ordering
            wn = []
            for q in range(NT // 2):
                wp = apool.tile([128, 2, S], FP8, tag=f"wn{q}")
                # first pair on DVE so the p1/silu pipeline starts earlier
                for r in range(2):
                    (nc.vector if q == 0 else nc.gpsimd).tensor_tensor(
                        out=_bview(wp[:, r, :], 0, [[16, 16], [1, 16]]),
                        in0=_bview(h[2 * q + r][:], 0, [[16, 16], [1, 16]]),
                        in1=_bview(rb[:], 0, [[1, 16], [16, 16]]),
                        op=MULT,
                    )
                wn.append(wp)
            warm(WARM["glu_p"])
            gts = []
            for ot in range(GT):
                p1 = ps_g.tile([128, S], F32, tag="pg")
                for q in range(NT // 2):
                    nc.tensor.matmul(
                        p1,
                        wv_sb[:, 2 * q : 2 * q + 2, ot * 128 : (ot + 1) * 128],
                        wn[q][:],
                        start=(q == 0),
                        stop=(q == NT // 2 - 1),
                        perf_mode=DR,
                    )
                s1 = apool.tile([128, S], FP8, tag="s1")
                if sim_compat:
                    # CoreSim has no Silu: emulate with Sigmoid + extra mul
                    sg = apool.tile([128, S], BF16, tag="sg")
                    nc.scalar.activation(sg, p1, AF.Sigmoid, scale=1.0 / WSC)
                    nc.vector.scalar_tensor_tensor(
                        out=s1, in0=p1, scalar=1.0 / WSC, in1=sg,
                        op0=MULT, op1=MULT,
                    )
                else:
                    nc.scalar.activation(s1, p1, AF.Silu, scale=1.0 / WSC)
                p3 = ps_g.tile([128, S], F32, tag="pg")
                for q in range(NT // 2):
                    nc.tensor.matmul(
                        p3,
                        wg_sb[:, 2 * q : 2 * q + 2, ot * 128 : (ot + 1) * 128],
                        wn[q][:],
                        start=(q == 0),
                        stop=(q == NT // 2 - 1),
                        perf_mode=DR,
                    )
                # gt = (p3 / WSC * GSC) * s1, stored fp8 (scaled by GSC)
                qg, rg_ = divmod(ot, 2)
                if rg_ == 0:
                    gp = gpool.tile([128, 2, S], FP8, tag=f"g{qg}")
                    gts.append(gp)
                nc.vector.scalar_tensor_tensor(
                    out=gts[qg][:, rg_, :], in0=p3, scalar=GSC / WSC, in1=s1,
                    op0=MULT, op1=MULT,
                )
            warm(WARM["glu_o"])
            last_accs = []
            for t in range(NT):
                po = ps_o.tile([128, S], F32, tag="po")
                # acc = (WSC*GSC)*h + (WSC*GSC)*update via a scaled identity;
                # h update is then a pure ACT copy with scale 1/(WSC*GSC)
                nc.tensor.matmul(
                    po,
                    id512[:],
                    h[t][:],
                    start=True,
                    stop=False,
                )
                for q in range(GT // 2):
                    nc.tensor.matmul(
                        po,
                        wo_sb[:, 2 * q : 2 * q + 2, t * 128 : (t + 1) * 128],
                        gts[q][:],
                        start=False,
                        stop=(q == GT // 2 - 1),
                        perf_mode=DR,
                    )
                if t < 2:
                    with nc.allow_low_precision(reason="f32r h tiles"):
                        nc.scalar.activation(
                            h[t], po, AF.Identity, scale=1.0 / (WSC * GSC)
                        )
                else:
                    with nc.allow_low_precision(reason="f32r h tiles"):
                        nc.vector.tensor_scalar_mul(
                            out=h[t], in0=po, scalar1=1.0 / (WSC * GSC)
                        )
                last_accs.append((po, 1.0 / (WSC * GSC)))

        # ---------- head ----------
        hw_sb = singles.tile([128, NT, VOC], BF16, tag="hw")
        nc.sync.dma_start(out=hw_sb, in_=hw_d.rearrange("(t p) v -> p t v", p=128))
        nc.tensor.ldweights(hw_sb[:, 0, 0:128])
        rb = rms_sbuf(last_accs)
        nrm = []
        for t in range(NT):
            n_ = apool.tile([128, S], BF16, tag=f"wn{t}")
            nc.vector.tensor_tensor(out=n_, in0=h[t], in1=rb, op=MULT)
            nrm.append(n_)
        for mc in range(VOC // 128):
            po = ps_o.tile([128, S], F32, tag="po")
            for kt in range(NT):
                nc.tensor.matmul(
                    po,
                    hw_sb[:, kt, mc * 128 : (mc + 1) * 128],
                    nrm[kt][:],
                    start=(kt == 0),
                    stop=(kt == NT - 1),
                )
            ot_sb = apool.tile([128, S], F32, tag="osb")
            nc.vector.tensor_copy(out=ot_sb, in_=po)
            nc.sync.dma_start(out=out_d[mc * 128 : (mc + 1) * 128, :], in_=ot_sb)

    nc.compile()
    return nc


def _prep_inputs(inputs, n_layers=L):
    """Host-side weight folding + layout prep. Returns dict of np arrays."""
    f = lambda k: np.asarray(inputs[k], dtype=np.float32)
    x = f("x")
    stem_w = f("stem_w")  # [H, CIN]
    rl, rg, rf = f("rms_local"), f("rms_global"), f("rms_ffn")
    al, ag, am = f("alpha_local"), f("alpha_global"), f("alpha_mlp")
    w_local, w_global = f("w_local"), f("w_global")  # [L, H, BLK, BLK]
    wv, wg, wo = f("wv"), f("wg"), f("wo")
    head_rms, head_w = f("head_rms"), f("head_w")
    hls = np.float32(np.asarray(inputs["head_logit_scale"]))

    bf = ml_dtypes.bfloat16
    nl = n_layers

    # local: fold alpha_local * rms_local[c] into Wl[c,p,j]; layout [c, 16p+j]
    wl_h = (w_local[:nl] * al[:nl, None, None, None] * rl[:nl, :, None, None]).reshape(
        nl, H, 256
    )
    # global: Wg[c,p,i]; layout [c, 16p+i]
    wm_h = (w_global[:nl] * ag[:nl, None, None, None] * rg[:nl, :, None, None]).reshape(
        nl, H, 256
    )
    # GLU: fold rms_ffn into wv/wg columns; alpha_mlp into wo
    wvT = np.ascontiguousarray(
        np.transpose(wv[:nl] * rf[:nl, None, :], (0, 2, 1))
    )  # [L, H, GLU]
    wgT = np.ascontiguousarray(np.transpose(wg[:nl] * rf[:nl, None, :], (0, 2, 1)))
    woT = np.ascontiguousarray(
        np.transpose(wo[:nl] * am[:nl, None, None], (0, 2, 1))
    )  # [L, GLU, H]
    headT = np.ascontiguousarray((head_w * head_rms[None, :] * hls).T)  # [H, VOC]

    stw_pad = np.zeros((384, H), np.float32)
    stw_pad[:CIN] = stem_w.T
    f8 = ml_dtypes.float8_e4m3
    common = {
        "stem_wT": stw_pad,  # [384, H] zero-padded
        "wvT": (wvT * 64.0).astype(f8),
        "wgT": (wgT * 64.0).astype(f8),
        "woT": (woT * 64.0).astype(f8),
        "wl": wl_h.astype(bf),
        "wm": wm_h.astype(bf),
        "headT": headT.astype(bf),
        "ident": np.eye(128, dtype=bf),
        "ident_r": np.eye(128, dtype=np.float32),
        "ident512": (512.0 * np.eye(128)).astype(np.float32),
        "ones_k": np.ones((128, 1), np.float32),
        "ones_m": np.ones((1, 128), np.float32),
    }
    per_core = []
    for b in range(B):
        xp = np.zeros((384, S), np.float32)
        xp[:CIN] = x[b, :, 0, :]
        per_core.append(dict(common, x=xp))
    return per_core


def run(inputs, n_layers=L, trace=False):
    key = n_layers
    if key not in _PROG_CACHE:
        _PROG_CACHE[key] = build_program(n_layers)
    nc = _PROG_CACHE[key]
    in_maps = _prep_inputs(inputs, n_layers)
    res = run_bass_kernel_spmd(nc, in_maps, core_ids=list(range(B)), trace=trace)
    out = np.stack([r["out"] for r in res.results])  # [B, VOC, S]
    return out[:, :, None, :].astype(np.float32), res


def kernel(**inputs):
    out, _ = run(inputs, L, trace=False)
    return out
